# revision 38
# baseline (speedup 1.0000x reference)
"""CFConv (SchNet-style continuous-filter convolution) Bass kernel for 8 trn2 cores.

Computation:  f = x@W1;  wf = w_ij * f[idx_j];  conv = segment_sum(wf, seg_i);
              out = conv@W2 + b2

Sharding: edges split equally across 8 cores at segment boundaries. Each core
computes the full node-feature table f = x@W1 (replicated), gathers neighbor
rows with dma_gather, multiplies by w_ij, segment-sums via one-hot matmuls on
the PE (PSUM accumulation over 128-atom windows), applies W2, and writes
per-window partial outputs. Host overlap-adds window outputs (exact: @W2 is
linear) and adds b2.

dma_gather indices are int16, so the f table is split by atom id at 32768
into two HBM tables; each core's edges are processed in two phases (A, B) —
segment-sum linearity makes the split exact. Each table is laid out
partition-major (see _remap_idx) so f-phase chunks write one contiguous
descriptor per partition, and table A completes early so phase-A gathers
overlap the rest of the f-phase. Within each segment window, edges are
sorted by gather row (the one-hot follows the edge, so order is free),
giving the gather ascending DMA addresses. Each block's gather is split
into 4 sub-gathers round-robined over 4 SWDGE queues to parallelize Q7
descriptor generation — the kernel's critical path.

Steady-state timing is measured by wrapping the program in a tc.For_i
hardware loop (bench_device repeat=64), amortizing the ~10ms host/axon
dispatch cost per jitted call.
"""

import math
import os
import sys

import numpy as np

for _p in ("/opt/trn_rl_repo", "/root/.axon_site/_ro/trn_rl_repo"):
    if os.path.isdir(_p) and _p not in sys.path:
        sys.path.insert(0, _p)

import ml_dtypes

BF16 = ml_dtypes.bfloat16
FP8 = ml_dtypes.float8_e4m3
W8 = os.environ.get("KERNEL_W8") == "1"  # stream w_ij as fp8e4m3

# Problem shape (hardcoded per harness contract)
N_ATOMS = 50000
N_EDGES = 1600000
D = 128
N_CORES = 8

TBL_SPLIT = 32768  # int16 gather-index limit

# Block geometry: GPW groups of 128 edges per PSUM window, WPB windows per block
GP = 128  # edges per group (matmul contraction dim)
WPB = 4  # windows per block (PSUM bank = 4*128 fp32 columns)

# dma_gather tuning (see exp_gather.py probes): descriptor generation on the
# Q7 SWDGE path is the kernel's critical path; split each block's gather
# across queues to parallelize generation.
GATHER_QUEUES = int(os.environ.get("GATHER_QUEUES", "4"))
GATHER_SPLIT = int(os.environ.get("GATHER_SPLIT", "4"))  # sub-gathers per block
DMA_SCRATCH = 16384

N_AP = math.ceil(N_ATOMS / 1024) * 1024  # padded atoms (1024-chunk f-phase)
NB_ROWS = N_AP - TBL_SPLIT  # table-B rows (atoms >= TBL_SPLIT)


def _remap_idx(a):
    """Atom id -> phase-local f-table row (partition-major within each table).

    Table A holds atoms < TBL_SPLIT, table B the rest; each is written
    partition-major (atom x at partition x%128, column x//128 of its table)
    so f-phase chunks land as one contiguous descriptor per partition and
    table A completes before table B starts.
    """
    a = np.asarray(a)
    in_a = a < TBL_SPLIT
    al = np.where(in_a, a, a - TBL_SPLIT)
    nr = np.where(in_a, TBL_SPLIT // 128, NB_ROWS // 128)
    return np.where(in_a, 0, TBL_SPLIT) + (al % 128) * nr + al // 128


def _pick_gpw(spans_ok, phase):
    # spans_ok(gpw, phase) -> bool; prefer big blocks (bounded by SBUF)
    for gpw in (12, 10, 8, 7, 6, 5, 4, 2, 1):
        if spans_ok(gpw, phase):
            return gpw
    raise ValueError("cannot window edges: segment spans too wide even at gpw=1")


def _core_edge_cuts(seg):
    """Split edges into N_CORES ranges at segment boundaries, near-equal sizes."""
    E = len(seg)
    cuts = [0]
    for k in range(1, N_CORES):
        t = k * E // N_CORES
        a = seg[t]
        cut = int(np.searchsorted(seg, a, side="left"))
        cuts.append(max(cut, cuts[-1]))
    cuts.append(E)
    return cuts


def _prep_phase(w, idx_local, seg, gpw):
    """Build device arrays for one (core, phase) edge list.

    w: [n,128] float32 edge filters, idx_local: [n] int64 table-local gather
    rows, seg: [n] int64 global atom ids (sorted). Returns dict with per-block
    tiled arrays, or None if a window span exceeds 128.
    """
    groups = gpw * WPB
    blk = groups * GP
    n = len(seg)
    nblk = max(1, math.ceil(n / blk))
    npad = nblk * blk

    w_pad = np.zeros((npad, D), dtype=np.float32)
    w_pad[:n] = w
    idx_pad = np.zeros(npad, dtype=np.int64)
    idx_pad[:n] = idx_local
    seg_pad = np.zeros(npad, dtype=np.int64)
    seg_pad[:n] = seg

    # window bases + local atom ids
    win_edges = gpw * GP
    nwin = nblk * WPB
    seg_w = seg_pad.reshape(nwin, win_edges)
    bases = seg_w[:, 0].copy()
    # pad tail of the partial window: give pads the window's base so c=0
    if n < npad:
        w_first = n // win_edges
        if n % win_edges:
            base_partial = seg_pad[w_first * win_edges]
            bases[w_first] = base_partial
            seg_pad[n : (w_first + 1) * win_edges] = base_partial
        # fully-padded windows already have seg=0, base=0
    c = seg_pad - np.repeat(bases, win_edges)
    if npad and (c.max() >= 128 or c.min() < 0):
        return None

    # within each window the segment one-hot follows the edge, so edge order
    # is free: sort by gather row for ascending DMA addresses
    for wi in range(nwin):
        sl = slice(wi * win_edges, (wi + 1) * win_edges)
        order = np.argsort(idx_pad[sl], kind="stable")
        w_pad[sl] = w_pad[sl][order]
        idx_pad[sl] = idx_pad[sl][order]
        c[sl] = c[sl][order]

    # tile layouts
    # edge i of block at [i%128 partition, i//128 group]
    wt = (
        w_pad.astype(BF16)
        .reshape(nblk, groups, GP, D)
        .transpose(0, 2, 1, 3)
        .copy()
    )  # [nblk, 128, groups, 128]
    ct = c.astype(BF16).reshape(nblk, groups, GP).transpose(0, 2, 1).copy()
    # idx wrapped: position i = s*16 + p -> [p, s]
    it = (
        idx_pad.astype(np.int16)
        .reshape(nblk, blk // 16, 16)
        .transpose(0, 2, 1)
    )  # [nblk, 16, blk//16]
    it = np.tile(it, (1, 8, 1)).copy()  # replicate to 128 partitions
    bases = bases.reshape(nblk, WPB)
    return dict(wt=wt, ct=ct, it=it, bases=bases, nblk=nblk)


def _zero_blocks(nblk, gpw):
    groups = gpw * WPB
    blk = groups * GP
    return dict(
        wt=np.zeros((nblk, GP, groups, D), dtype=BF16),
        ct=np.zeros((nblk, GP, groups), dtype=BF16),
        it=np.zeros((nblk, 128, blk // 16), dtype=np.int16),
        bases=np.zeros((nblk, WPB), dtype=np.int64),
        nblk=nblk,
    )


def _pad_blocks(ph, nblk, gpw):
    if ph["nblk"] == nblk:
        return ph
    z = _zero_blocks(nblk - ph["nblk"], gpw)
    return dict(
        wt=np.concatenate([ph["wt"], z["wt"]]),
        ct=np.concatenate([ph["ct"], z["ct"]]),
        it=np.concatenate([ph["it"], z["it"]]),
        bases=np.concatenate([ph["bases"], z["bases"]]),
        nblk=nblk,
    )


def prep_inputs(x, w_ij, seg_i, idx_j, W1, W2):
    """Host-side preparation. Returns (per_core_maps, shared, plan)."""
    seg = np.asarray(seg_i, dtype=np.int64)
    idx = np.asarray(idx_j, dtype=np.int64)
    w = np.asarray(w_ij, dtype=np.float32)
    x = np.asarray(x, dtype=np.float32)

    idx2 = _remap_idx(idx)  # f-table rows (partition-major layout)

    cuts = _core_edge_cuts(seg)

    def spans_ok(gpw, phase):
        for k in range(N_CORES):
            lo, hi = cuts[k], cuts[k + 1]
            m = idx2[lo:hi] < TBL_SPLIT
            sel = m if phase == 0 else ~m
            s = seg[lo:hi][sel]
            nw = math.ceil(len(s) / (gpw * GP))
            for wi in range(nw):
                ss = s[wi * gpw * GP : (wi + 1) * gpw * GP]
                if len(ss) and ss[-1] - ss[0] >= 128:
                    return False
        return True

    gpw_a = _pick_gpw(spans_ok, 0)
    gpw_b = _pick_gpw(spans_ok, 1)

    phases = []  # [core][phase] dicts
    for k in range(N_CORES):
        lo, hi = cuts[k], cuts[k + 1]
        m = idx2[lo:hi] < TBL_SPLIT
        pair = []
        for pi, sel in enumerate((m, ~m)):
            e = np.nonzero(sel)[0] + lo
            ph = _prep_phase(
                w[e],
                idx2[e] - (0 if pi == 0 else TBL_SPLIT),
                seg[e],
                gpw_a if pi == 0 else gpw_b,
            )
            assert ph is not None, "span check passed but prep failed"
            pair.append(ph)
        phases.append(pair)

    nblk_a = max(p[0]["nblk"] for p in phases)
    nblk_b = max(p[1]["nblk"] for p in phases)
    nblk = nblk_a + nblk_b

    def _aux_pack(ct, it, groups, blk):
        n = ct.shape[0]
        ab = 2 * groups + blk // 8
        aux = np.zeros((n, 128, ab), dtype=np.uint8)
        aux[:, :, : 2 * groups] = ct.view(np.uint8).reshape(n, 128, -1)
        aux[:, :, 2 * groups :] = it.view(np.uint8).reshape(n, 128, -1)
        return aux

    per_core = []
    all_bases = []
    for k in range(N_CORES):
        pa = _pad_blocks(phases[k][0], nblk_a, gpw_a)
        pb = _pad_blocks(phases[k][1], nblk_b, gpw_b)
        wdt = FP8 if W8 else BF16
        per_core.append(
            dict(
                wt_a=np.ascontiguousarray(pa["wt"].astype(wdt)),
                wt_b=np.ascontiguousarray(pb["wt"].astype(wdt)),
                aux_a=_aux_pack(
                    pa["ct"], pa["it"], gpw_a * WPB, gpw_a * WPB * GP
                ),
                aux_b=_aux_pack(
                    pb["ct"], pb["it"], gpw_b * WPB, gpw_b * WPB * GP
                ),
            )
        )
        all_bases.append(np.concatenate([pa["bases"], pb["bases"]]))

    # shared tensors
    xT = np.zeros((D, N_AP), dtype=BF16)
    xT[:, :N_ATOMS] = x.T.astype(BF16)
    iota = np.broadcast_to(np.arange(GP, dtype=np.float32), (GP, GP)).astype(BF16)
    shared = dict(
        xT=np.ascontiguousarray(xT),
        W1=W1.astype(BF16),
        W2=W2.astype(np.float32),
        iota=np.ascontiguousarray(iota),
    )
    plan = dict(
        gpw_a=gpw_a,
        gpw_b=gpw_b,
        nblk_a=nblk_a,
        nblk_b=nblk_b,
        nblk=nblk,
        bases=all_bases,
    )
    return per_core, shared, plan


def host_combine(stages, plan, b2):
    """stages: list of [NBLK, 128, WPB*128] bf16 outT arrays (per core)."""
    out = np.zeros((N_ATOMS + GP, D), dtype=np.float64)
    for k in range(N_CORES):
        st = np.asarray(stages[k]).astype(np.float64)
        nblk = plan["nblk"]
        # [NBLK, 128do, WPB, 128a] -> [NBLK, WPB, 128a, 128do]
        st = st.reshape(nblk, D, WPB, GP).transpose(0, 2, 3, 1)
        bases = plan["bases"][k]
        for b in range(nblk):
            for wi in range(WPB):
                base = int(bases[b, wi])
                out[base : base + GP] += st[b, wi]
    return (out[:N_ATOMS] + np.asarray(b2, dtype=np.float64)).astype(np.float32)


# ---------------------------------------------------------------------------
# numpy emulation of the device program (for validating the decomposition)
# ---------------------------------------------------------------------------


def emulate_device(per_core, shared, plan, exact=False):
    cast = (lambda a: a.astype(np.float32)) if exact else (
        lambda a: a.astype(BF16).astype(np.float32)
    )
    xT = shared["xT"].astype(np.float32)
    W1 = shared["W1"].astype(np.float32)
    W2 = shared["W2"].astype(np.float32)
    f = cast(xT.T @ W1)  # [N_AP, 128] in atom order (bf16-rounded)
    # partition-major table: row r = (a%128)*NROWS + a//128  ->  f2[r] = f[a]
    a_of_r = np.empty(N_AP, dtype=np.int64)
    r = _remap_idx(np.arange(N_AP))
    a_of_r[r] = np.arange(N_AP)
    f2 = f[a_of_r]
    stages = []
    for k in range(N_CORES):
        m = per_core[k]
        nblk = plan["nblk"]
        stage = np.zeros((nblk, D, WPB * GP), dtype=np.float32)
        for b in range(nblk):
            in_a = b < plan["nblk_a"]
            gpw = plan["gpw_a"] if in_a else plan["gpw_b"]
            groups = gpw * WPB
            blk = groups * GP
            aux = m["aux_a"] if in_a else m["aux_b"]
            wt = m["wt_a"] if in_a else m["wt_b"]
            bl = b if in_a else b - plan["nblk_a"]
            tbl_off = 0 if in_a else TBL_SPLIT
            ct = (
                aux[bl, :, : 2 * groups].copy().view(BF16).astype(np.float32)
            )  # [128, groups]
            it = aux[bl, :, 2 * groups :].copy().view(np.int16)
            idx = it[:16].T.reshape(-1).astype(np.int64)  # [blk] in (s p) order
            w_t = wt[bl].astype(np.float32)  # [128, groups, 128]
            fj = f2[idx + tbl_off].reshape(groups, GP, D).transpose(1, 0, 2)
            wf = cast(w_t * fj)  # [128, groups, 128]
            convT = np.zeros((D, WPB * GP), dtype=np.float32)
            for g in range(groups):
                S = (ct[:, g : g + 1] == np.arange(GP)[None, :]).astype(np.float32)
                wi = g // gpw
                convT[:, wi * GP : (wi + 1) * GP] += wf[:, g, :].T @ S
            stage[b] = cast(W2.T @ convT)
        stages.append(stage.astype(BF16))
    return stages


# ---------------------------------------------------------------------------
# bass device program
# ---------------------------------------------------------------------------


def build_program(plan, repeat=1):
    """Build the device program. With repeat>1 the whole computation runs
    `repeat` times inside a hardware loop (identical work each iteration;
    outputs are rewritten idempotently) so steady-state per-execution time
    can be measured as exec_time/repeat, amortizing host dispatch cost."""
    from contextlib import nullcontext

    import concourse.bacc as bacc
    import concourse.mybir as mybir
    import concourse.tile as tile

    fp32 = mybir.dt.float32
    bf16 = mybir.dt.bfloat16
    i16 = mybir.dt.int16
    u8 = mybir.dt.uint8

    gpw_a = plan["gpw_a"]
    gpw_b = plan["gpw_b"]
    nblk = plan["nblk"]
    nblk_a = plan["nblk_a"]
    nblk_b = plan["nblk_b"]

    def _geom(gpw):
        groups = gpw * WPB
        blk = groups * GP
        return groups, blk, 2 * groups + blk // 8

    groups_a, blk_a, ab_a = _geom(gpw_a)
    groups_b, blk_b, ab_b = _geom(gpw_b)

    nc = bacc.Bacc(
        "TRN2",
        target_bir_lowering=False,
        debug=False,
        num_devices=N_CORES,
        num_swdge_queues=GATHER_QUEUES,
        dynamic_dma_scratch_size=DMA_SCRATCH,
    )

    xT_d = nc.dram_tensor("xT", [D, N_AP], bf16, kind="ExternalInput")
    W1_d = nc.dram_tensor("W1", [D, D], bf16, kind="ExternalInput")
    W2_d = nc.dram_tensor("W2", [D, D], fp32, kind="ExternalInput")
    iota_d = nc.dram_tensor("iota", [GP, GP], bf16, kind="ExternalInput")
    wdt = mybir.dt.float8e4 if W8 else bf16
    wta_d = nc.dram_tensor(
        "wt_a", [nblk_a, GP, groups_a, D], wdt, kind="ExternalInput"
    )
    wtb_d = nc.dram_tensor(
        "wt_b", [nblk_b, GP, groups_b, D], wdt, kind="ExternalInput"
    )
    auxa_d = nc.dram_tensor(
        "aux_a", [nblk_a, 128, ab_a], u8, kind="ExternalInput"
    )
    auxb_d = nc.dram_tensor(
        "aux_b", [nblk_b, 128, ab_b], u8, kind="ExternalInput"
    )
    stage_d = nc.dram_tensor(
        "stage", [nblk, D, WPB * GP], bf16, kind="ExternalOutput"
    )

    with tile.TileContext(nc) as tc:
        with (
            tc.tile_pool(name="consts", bufs=1) as consts,
            tc.tile_pool(name="dram", bufs=1, space="DRAM") as dram_pool,
        ):
            # per-phase f tables, each partition-major (see _remap_idx)
            f_da = dram_pool.tile([TBL_SPLIT, D], bf16)
            f_db = dram_pool.tile([NB_ROWS, D], bf16)
            f_pma = f_da[:].rearrange("(p c) d -> p c d", p=128)
            f_pmb = f_db[:].rearrange("(p c) d -> p c d", p=128)

            W1_sb = consts.tile([D, D], bf16)
            nc.sync.dma_start(W1_sb[:], W1_d[:])
            W2_sb = consts.tile([D, D], fp32)
            nc.sync.dma_start(W2_sb[:], W2_d[:])
            iota_sb = consts.tile([GP, GP], bf16)
            nc.sync.dma_start(iota_sb[:], iota_d[:])

            _ab_nofphase = os.environ.get("KERNEL_NOFPHASE") == "1"
            _ab_nogather = os.environ.get("KERNEL_NOGATHER") == "1"
            _ab_gatheronly = os.environ.get("KERNEL_GATHERONLY") == "1"

            rep_ctx = tc.For_i(0, repeat) if repeat > 1 else nullcontext(0)
            with rep_ctx:
                # ---------------- f-phase: f = x @ W1 ----------------
                CH = 8  # 128-atom tiles per chunk
                nchunks = N_AP // (CH * GP)
                chunk_list = (
                    [0, TBL_SPLIT // (CH * GP)]
                    if _ab_nofphase
                    else range(nchunks)
                )
                with (
                    tc.tile_pool(name="wsb", bufs=4) as w_pool,
                    tc.tile_pool(name="fj", bufs=4) as fj_pool,
                    tc.tile_pool(name="wf", bufs=2) as wf_pool,
                    tc.tile_pool(name="S", bufs=2) as s_pool,
                    tc.tile_pool(name="aux", bufs=6) as aux_pool,
                    tc.tile_pool(name="cvs", bufs=2) as cvs_pool,
                    tc.tile_pool(name="os", bufs=2) as os_pool,
                    tc.tile_pool(name="cvp", bufs=2, space="PSUM") as cvp_pool,
                    tc.tile_pool(name="otp", bufs=2, space="PSUM") as otp_pool,
                    tc.tile_pool(name="xt", bufs=3) as xt_pool,
                    tc.tile_pool(name="fsb", bufs=3) as fsb_pool,
                    tc.tile_pool(name="fps", bufs=2, space="PSUM") as fps_pool,
                ):
                    # prefetch the first blocks' inputs ahead of the f-phase
                    # DMA queue so the first gathers fire as soon as table A
                    # is written
                    PREF = 3
                    pre = []
                    for b in range(min(PREF, nblk_a)):
                        aux_sb = aux_pool.tile([128, ab_a], u8)
                        nc.sync.dma_start(aux_sb[:], auxa_d[b])
                        w_sb = w_pool.tile([GP, groups_a, D], wdt)
                        nc.sync.dma_start(w_sb[:], wta_d[b])
                        pre.append((aux_sb, w_sb))

                    for ci in chunk_list:
                        a0 = ci * CH * GP
                        xt = xt_pool.tile([D, CH * GP], bf16)
                        nc.sync.dma_start(xt[:], xT_d[:, a0 : a0 + CH * GP])
                        fps = fps_pool.tile([GP, CH, D], fp32)
                        for i in range(CH):
                            nc.tensor.matmul(
                                fps[:, i, :],
                                xt[:, i * GP : (i + 1) * GP],
                                W1_sb[:],
                                start=True,
                                stop=True,
                            )
                        fsb = fsb_pool.tile([GP, CH, D], bf16)
                        nc.scalar.copy(fsb[:], fps[:])
                        # atom a0+i*128+p -> table row (p, local_col): one
                        # contiguous descriptor per partition
                        ca = TBL_SPLIT // (CH * GP)
                        dst = (
                            f_pma[:, ci * CH : (ci + 1) * CH, :]
                            if ci < ca
                            else f_pmb[:, (ci - ca) * CH : (ci - ca + 1) * CH, :]
                        )
                        nc.sync.dma_start(dst, fsb[:])

                    # ---------------- main loop ----------------
                    gq = 0
                    for b in range(nblk):
                        in_a = b < nblk_a
                        gpw = gpw_a if in_a else gpw_b
                        groups = gpw * WPB
                        blk = groups * GP
                        ab = ab_a if in_a else ab_b
                        bl = b if in_a else b - nblk_a
                        wt_d = wta_d if in_a else wtb_d
                        aux_d = auxa_d if in_a else auxb_d

                        if b < len(pre):
                            aux_sb, w_sb = pre[b]
                        else:
                            aux_sb = aux_pool.tile([128, ab], u8)
                            nc.sync.dma_start(aux_sb[:], aux_d[bl])
                            w_sb = w_pool.tile([GP, groups, D], wdt)
                            nc.sync.dma_start(w_sb[:], wt_d[bl])
                        ct_sb = aux_sb[:, : 2 * groups].bitcast(bf16)
                        it_sb = aux_sb[:, 2 * groups :].bitcast(i16)

                        fj_sb = fj_pool.tile([GP, groups, D], bf16)
                        tbl = f_da[:] if in_a else f_db[:]
                        if _ab_nogather:
                            nc.vector.memset(fj_sb[:, 0, :], 0.0)
                        else:
                            ns = GATHER_SPLIT
                            n_i = blk // ns
                            for h in range(ns):
                                nc.gpsimd.dma_gather(
                                    fj_sb[:, h * (groups // ns) :
                                          (h + 1) * (groups // ns), :],
                                    tbl,
                                    it_sb[:, h * (n_i // 16) :
                                          (h + 1) * (n_i // 16)],
                                    n_i,
                                    n_i,
                                    D,
                                    single_packet=False,
                                    queue_num=gq % GATHER_QUEUES,
                                )
                                gq += 1

                        if _ab_gatheronly:
                            osb = os_pool.tile([D, WPB * GP], bf16)
                            nc.vector.memset(osb[:, 0:4], 0.0)
                            nc.sync.dma_start(stage_d[b], osb[:])
                            continue

                        wf_sb = wf_pool.tile([GP, groups, D], bf16)
                        nc.vector.tensor_mul(wf_sb[:], w_sb[:], fj_sb[:])

                        s_sb = s_pool.tile([GP, groups, D], bf16)
                        nc.vector.tensor_tensor(
                            s_sb[:],
                            ct_sb.unsqueeze(2).broadcast_to([GP, groups, GP]),
                            iota_sb[:].unsqueeze(1).broadcast_to(
                                [GP, groups, GP]
                            ),
                            mybir.AluOpType.is_equal,
                        )

                        cvp = cvp_pool.tile([D, WPB, GP], fp32)
                        for g in range(groups):
                            wi = g // gpw
                            nc.tensor.matmul(
                                cvp[:, wi, :],
                                wf_sb[:, g, :],
                                s_sb[:, g, :],
                                start=(g % gpw == 0),
                                stop=(g % gpw == gpw - 1),
                            )
                        cvs = cvs_pool.tile([D, WPB * GP], fp32)
                        nc.scalar.copy(
                            cvs[:], cvp[:].rearrange("d w a -> d (w a)")
                        )

                        otp = otp_pool.tile([D, WPB * GP], fp32)
                        nc.tensor.matmul(
                            otp[:], W2_sb[:], cvs[:], start=True, stop=True
                        )
                        osb = os_pool.tile([D, WPB * GP], bf16)
                        nc.scalar.copy(osb[:], otp[:])
                        nc.sync.dma_start(stage_d[b], osb[:])

    nc.compile()
    return nc


def run_device(per_core, shared, plan, trace=False):
    from concourse import bass_utils

    nc = build_program(plan)
    in_maps = []
    for k in range(N_CORES):
        m = dict(shared)
        m.update(per_core[k])
        in_maps.append(
            {
                "xT": np.ascontiguousarray(m["xT"]),
                "W1": np.ascontiguousarray(m["W1"]),
                "W2": np.ascontiguousarray(m["W2"]),
                "iota": np.ascontiguousarray(m["iota"]),
                "wt_a": np.ascontiguousarray(m["wt_a"]),
                "wt_b": np.ascontiguousarray(m["wt_b"]),
                "aux_a": np.ascontiguousarray(m["aux_a"]),
                "aux_b": np.ascontiguousarray(m["aux_b"]),
            }
        )
    res = bass_utils.run_bass_kernel_spmd(
        nc, in_maps, core_ids=list(range(N_CORES)), trace=trace
    )
    stages = [r["stage"] for r in res.results]
    return stages, res


def bench_device(per_core, shared, plan, repeat=64, nbatches=4):
    """Steady-state per-execution device time.

    The device program repeats the full computation `repeat` times inside a
    hardware loop (see build_program), so one NEFF execution performs
    `repeat` back-to-back runs. Per-run time = call_time / repeat; the ~10ms
    host/axon dispatch overhead amortizes to noise.
    """
    import time

    import jax
    from jax.sharding import Mesh, PartitionSpec
    from jax.experimental.shard_map import shard_map
    from concourse.bass2jax import (
        _bass_exec_p,
        install_neuronx_cc_hook,
        partition_id_tensor,
    )
    import concourse.mybir as mybir

    install_neuronx_cc_hook()
    nc = build_program(plan, repeat=repeat)
    partition_name = (
        nc.partition_id_tensor.name if nc.partition_id_tensor else None
    )

    in_names = []
    out_names = []
    out_avals = []
    zero_outs = []
    for alloc in nc.m.functions[0].allocations:
        if not isinstance(alloc, mybir.MemoryLocationSet):
            continue
        name = alloc.memorylocations[0].name
        if alloc.kind == "ExternalInput":
            if name != partition_name:
                in_names.append(name)
        elif alloc.kind == "ExternalOutput":
            out_names.append(name)
            dt = mybir.dt.np(alloc.dtype)
            out_avals.append(
                jax.core.ShapedArray(tuple(alloc.tensor_shape), dt)
            )
            zero_outs.append(np.zeros(tuple(alloc.tensor_shape), dt))
    n_params = len(in_names)
    all_names = in_names + out_names
    if partition_name is not None:
        all_names = all_names + [partition_name]

    def _body(*args):
        operands = list(args)
        if partition_name is not None:
            operands.append(partition_id_tensor())
        outs = _bass_exec_p.bind(
            *operands,
            out_avals=tuple(out_avals),
            in_names=tuple(all_names),
            out_names=tuple(out_names),
            lowering_input_output_aliases=(),
            sim_require_finite=True,
            sim_require_nnan=True,
            nc=nc,
        )
        return tuple(outs)

    devices = jax.devices()[:N_CORES]
    mesh = Mesh(np.asarray(devices), ("core",))
    nin = n_params + len(zero_outs)
    sharded = jax.jit(
        shard_map(
            _body,
            mesh=mesh,
            in_specs=(PartitionSpec("core"),) * nin,
            out_specs=(PartitionSpec("core"),) * len(out_names),
            check_rep=False,
        ),
        keep_unused=True,
    )

    in_maps = []
    for k in range(N_CORES):
        m = dict(shared)
        m.update(per_core[k])
        in_maps.append(m)
    concat = [
        np.concatenate([np.asarray(in_maps[c][n]) for c in range(N_CORES)], axis=0)
        for n in in_names
    ] + [np.zeros((N_CORES * z.shape[0], *z.shape[1:]), z.dtype) for z in zero_outs]
    from jax.sharding import NamedSharding

    sh = NamedSharding(mesh, PartitionSpec("core"))
    dev_in = [jax.device_put(a, sh) for a in concat]

    # warmup (compile + first run)
    out = sharded(*dev_in)
    jax.block_until_ready(out)
    t0 = time.perf_counter()
    out = sharded(*dev_in)
    jax.block_until_ready(out)
    single = (time.perf_counter() - t0) / repeat
    times = []
    for _ in range(nbatches):
        tb = time.perf_counter()
        out2 = sharded(*dev_in)
        jax.block_until_ready(out2)
        times.append((time.perf_counter() - tb) / repeat)
    per_iter = min(times)
    stage_g = np.asarray(out[0]).reshape(N_CORES, *out_avals[0].shape)
    stages = [stage_g[c] for c in range(N_CORES)]
    return stages, dict(single_s=single, per_iter_s=per_iter)


def kernel(x, w_ij, seg_i, idx_j, seg_i_sum, W1, W2, b2, _trace=False, _emulate=False):
    per_core, shared, plan = prep_inputs(x, w_ij, seg_i, idx_j, W1, W2)
    if _emulate:
        stages = emulate_device(per_core, shared, plan)
        res = None
    else:
        stages, res = run_device(per_core, shared, plan, trace=_trace)
    out = host_combine(stages, plan, b2)
    if _trace:
        return out, res
    return out


# revision 40
# speedup vs baseline: 1.0481x; 1.0481x over previous
"""CFConv (SchNet-style continuous-filter convolution) Bass kernel for 8 trn2 cores.

Computation:  f = x@W1;  wf = w_ij * f[idx_j];  conv = segment_sum(wf, seg_i);
              out = conv@W2 + b2

Sharding: edges split equally across 8 cores at segment boundaries. Each core
computes the full node-feature table f = x@W1 (replicated), gathers neighbor
rows with dma_gather, multiplies by w_ij, segment-sums via one-hot matmuls on
the PE (PSUM accumulation over 128-atom windows), applies W2, and writes
per-window partial outputs. Host overlap-adds window outputs (exact: @W2 is
linear) and adds b2.

dma_gather indices are int16, so the f table is split by atom id at 32768
into two HBM tables; each core's edges are processed in two phases (A, B) —
segment-sum linearity makes the split exact. Each table is laid out
partition-major (see _remap_idx) so f-phase chunks write one contiguous
descriptor per partition, and table A completes early so phase-A gathers
overlap the rest of the f-phase. Within each segment window, edges are
sorted by gather row (the one-hot follows the edge, so order is free),
giving the gather ascending DMA addresses. Each block's gather is split
into 4 sub-gathers round-robined over 4 SWDGE queues to parallelize Q7
descriptor generation — the kernel's critical path.

Steady-state timing is measured by wrapping the program in a tc.For_i
hardware loop (bench_device repeat=64), amortizing the ~10ms host/axon
dispatch cost per jitted call.
"""

import math
import os
import sys

import numpy as np

for _p in ("/opt/trn_rl_repo", "/root/.axon_site/_ro/trn_rl_repo"):
    if os.path.isdir(_p) and _p not in sys.path:
        sys.path.insert(0, _p)

import ml_dtypes

BF16 = ml_dtypes.bfloat16
FP8 = ml_dtypes.float8_e4m3
W8 = os.environ.get("KERNEL_W8") == "1"  # stream w_ij as fp8e4m3

# Problem shape (hardcoded per harness contract)
N_ATOMS = 50000
N_EDGES = 1600000
D = 128
N_CORES = 8

TBL_SPLIT = 32768  # int16 gather-index limit

# Block geometry: GPW groups of 128 edges per PSUM window, WPB windows per block
GP = 128  # edges per group (matmul contraction dim)
WPB = 4  # windows per block (PSUM bank = 4*128 fp32 columns)

# dma_gather tuning (see exp_gather.py probes): descriptor generation on the
# Q7 SWDGE path is the kernel's critical path; split each block's gather
# across queues to parallelize generation.
GATHER_QUEUES = int(os.environ.get("GATHER_QUEUES", "4"))
GATHER_SPLIT = int(os.environ.get("GATHER_SPLIT", "4"))  # sub-gathers per block
DMA_SCRATCH = int(os.environ.get("DMA_SCRATCH", "24576"))

N_AP = math.ceil(N_ATOMS / 1024) * 1024  # padded atoms (1024-chunk f-phase)
NB_ROWS = N_AP - TBL_SPLIT  # table-B rows (atoms >= TBL_SPLIT)


def _remap_idx(a):
    """Atom id -> phase-local f-table row (partition-major within each table).

    Table A holds atoms < TBL_SPLIT, table B the rest; each is written
    partition-major (atom x at partition x%128, column x//128 of its table)
    so f-phase chunks land as one contiguous descriptor per partition and
    table A completes before table B starts.
    """
    a = np.asarray(a)
    in_a = a < TBL_SPLIT
    al = np.where(in_a, a, a - TBL_SPLIT)
    nr = np.where(in_a, TBL_SPLIT // 128, NB_ROWS // 128)
    return np.where(in_a, 0, TBL_SPLIT) + (al % 128) * nr + al // 128


def _pick_gpw(spans_ok, phase):
    # spans_ok(gpw, phase) -> bool; prefer big blocks (bounded by SBUF)
    for gpw in (12, 10, 8, 7, 6, 5, 4, 2, 1):
        if spans_ok(gpw, phase):
            return gpw
    raise ValueError("cannot window edges: segment spans too wide even at gpw=1")


def _core_edge_cuts(seg):
    """Split edges into N_CORES ranges at segment boundaries, near-equal sizes."""
    E = len(seg)
    cuts = [0]
    for k in range(1, N_CORES):
        t = k * E // N_CORES
        a = seg[t]
        cut = int(np.searchsorted(seg, a, side="left"))
        cuts.append(max(cut, cuts[-1]))
    cuts.append(E)
    return cuts


def _prep_phase(w, idx_local, seg, gpw):
    """Build device arrays for one (core, phase) edge list.

    w: [n,128] float32 edge filters, idx_local: [n] int64 table-local gather
    rows, seg: [n] int64 global atom ids (sorted). Returns dict with per-block
    tiled arrays, or None if a window span exceeds 128.
    """
    groups = gpw * WPB
    blk = groups * GP
    n = len(seg)
    nblk = max(1, math.ceil(n / blk))
    npad = nblk * blk

    w_pad = np.zeros((npad, D), dtype=np.float32)
    w_pad[:n] = w
    idx_pad = np.zeros(npad, dtype=np.int64)
    idx_pad[:n] = idx_local
    seg_pad = np.zeros(npad, dtype=np.int64)
    seg_pad[:n] = seg

    # window bases + local atom ids
    win_edges = gpw * GP
    nwin = nblk * WPB
    seg_w = seg_pad.reshape(nwin, win_edges)
    bases = seg_w[:, 0].copy()
    # pad tail of the partial window: give pads the window's base so c=0
    if n < npad:
        w_first = n // win_edges
        if n % win_edges:
            base_partial = seg_pad[w_first * win_edges]
            bases[w_first] = base_partial
            seg_pad[n : (w_first + 1) * win_edges] = base_partial
        # fully-padded windows already have seg=0, base=0
    c = seg_pad - np.repeat(bases, win_edges)
    if npad and (c.max() >= 128 or c.min() < 0):
        return None

    # within each window the segment one-hot follows the edge, so edge order
    # is free: sort by gather row for ascending DMA addresses
    for wi in range(nwin):
        sl = slice(wi * win_edges, (wi + 1) * win_edges)
        order = np.argsort(idx_pad[sl], kind="stable")
        w_pad[sl] = w_pad[sl][order]
        idx_pad[sl] = idx_pad[sl][order]
        c[sl] = c[sl][order]

    # tile layouts
    # edge i of block at [i%128 partition, i//128 group]
    wt = (
        w_pad.astype(BF16)
        .reshape(nblk, groups, GP, D)
        .transpose(0, 2, 1, 3)
        .copy()
    )  # [nblk, 128, groups, 128]
    ct = c.astype(BF16).reshape(nblk, groups, GP).transpose(0, 2, 1).copy()
    # idx wrapped: position i = s*16 + p -> [p, s]
    it = (
        idx_pad.astype(np.int16)
        .reshape(nblk, blk // 16, 16)
        .transpose(0, 2, 1)
    )  # [nblk, 16, blk//16]
    it = np.tile(it, (1, 8, 1)).copy()  # replicate to 128 partitions
    bases = bases.reshape(nblk, WPB)
    return dict(wt=wt, ct=ct, it=it, bases=bases, nblk=nblk)


def _zero_blocks(nblk, gpw):
    groups = gpw * WPB
    blk = groups * GP
    return dict(
        wt=np.zeros((nblk, GP, groups, D), dtype=BF16),
        ct=np.zeros((nblk, GP, groups), dtype=BF16),
        it=np.zeros((nblk, 128, blk // 16), dtype=np.int16),
        bases=np.zeros((nblk, WPB), dtype=np.int64),
        nblk=nblk,
    )


def _pad_blocks(ph, nblk, gpw):
    if ph["nblk"] == nblk:
        return ph
    z = _zero_blocks(nblk - ph["nblk"], gpw)
    return dict(
        wt=np.concatenate([ph["wt"], z["wt"]]),
        ct=np.concatenate([ph["ct"], z["ct"]]),
        it=np.concatenate([ph["it"], z["it"]]),
        bases=np.concatenate([ph["bases"], z["bases"]]),
        nblk=nblk,
    )


def prep_inputs(x, w_ij, seg_i, idx_j, W1, W2):
    """Host-side preparation. Returns (per_core_maps, shared, plan)."""
    seg = np.asarray(seg_i, dtype=np.int64)
    idx = np.asarray(idx_j, dtype=np.int64)
    w = np.asarray(w_ij, dtype=np.float32)
    x = np.asarray(x, dtype=np.float32)

    idx2 = _remap_idx(idx)  # f-table rows (partition-major layout)

    cuts = _core_edge_cuts(seg)

    def spans_ok(gpw, phase):
        for k in range(N_CORES):
            lo, hi = cuts[k], cuts[k + 1]
            m = idx2[lo:hi] < TBL_SPLIT
            sel = m if phase == 0 else ~m
            s = seg[lo:hi][sel]
            nw = math.ceil(len(s) / (gpw * GP))
            for wi in range(nw):
                ss = s[wi * gpw * GP : (wi + 1) * gpw * GP]
                if len(ss) and ss[-1] - ss[0] >= 128:
                    return False
        return True

    gpw_a = _pick_gpw(spans_ok, 0)
    gpw_b = _pick_gpw(spans_ok, 1)

    phases = []  # [core][phase] dicts
    for k in range(N_CORES):
        lo, hi = cuts[k], cuts[k + 1]
        m = idx2[lo:hi] < TBL_SPLIT
        pair = []
        for pi, sel in enumerate((m, ~m)):
            e = np.nonzero(sel)[0] + lo
            ph = _prep_phase(
                w[e],
                idx2[e] - (0 if pi == 0 else TBL_SPLIT),
                seg[e],
                gpw_a if pi == 0 else gpw_b,
            )
            assert ph is not None, "span check passed but prep failed"
            pair.append(ph)
        phases.append(pair)

    nblk_a = max(p[0]["nblk"] for p in phases)
    nblk_b = max(p[1]["nblk"] for p in phases)
    nblk = nblk_a + nblk_b

    def _aux_pack(ct, it, groups, blk):
        n = ct.shape[0]
        ab = 2 * groups + blk // 8
        aux = np.zeros((n, 128, ab), dtype=np.uint8)
        aux[:, :, : 2 * groups] = ct.view(np.uint8).reshape(n, 128, -1)
        aux[:, :, 2 * groups :] = it.view(np.uint8).reshape(n, 128, -1)
        return aux

    per_core = []
    all_bases = []
    for k in range(N_CORES):
        pa = _pad_blocks(phases[k][0], nblk_a, gpw_a)
        pb = _pad_blocks(phases[k][1], nblk_b, gpw_b)
        wdt = FP8 if W8 else BF16
        per_core.append(
            dict(
                wt_a=np.ascontiguousarray(pa["wt"].astype(wdt)),
                wt_b=np.ascontiguousarray(pb["wt"].astype(wdt)),
                aux_a=_aux_pack(
                    pa["ct"], pa["it"], gpw_a * WPB, gpw_a * WPB * GP
                ),
                aux_b=_aux_pack(
                    pb["ct"], pb["it"], gpw_b * WPB, gpw_b * WPB * GP
                ),
            )
        )
        all_bases.append(np.concatenate([pa["bases"], pb["bases"]]))

    # shared tensors
    xT = np.zeros((D, N_AP), dtype=BF16)
    xT[:, :N_ATOMS] = x.T.astype(BF16)
    iota = np.broadcast_to(np.arange(GP, dtype=np.float32), (GP, GP)).astype(BF16)
    shared = dict(
        xT=np.ascontiguousarray(xT),
        W1=W1.astype(BF16),
        W2=W2.astype(np.float32),
        iota=np.ascontiguousarray(iota),
    )
    plan = dict(
        gpw_a=gpw_a,
        gpw_b=gpw_b,
        nblk_a=nblk_a,
        nblk_b=nblk_b,
        nblk=nblk,
        bases=all_bases,
    )
    return per_core, shared, plan


def host_combine(stages, plan, b2):
    """stages: list of [NBLK, 128, WPB*128] bf16 outT arrays (per core)."""
    out = np.zeros((N_ATOMS + GP, D), dtype=np.float64)
    for k in range(N_CORES):
        st = np.asarray(stages[k]).astype(np.float64)
        nblk = plan["nblk"]
        # [NBLK, 128do, WPB, 128a] -> [NBLK, WPB, 128a, 128do]
        st = st.reshape(nblk, D, WPB, GP).transpose(0, 2, 3, 1)
        bases = plan["bases"][k]
        for b in range(nblk):
            for wi in range(WPB):
                base = int(bases[b, wi])
                out[base : base + GP] += st[b, wi]
    return (out[:N_ATOMS] + np.asarray(b2, dtype=np.float64)).astype(np.float32)


# ---------------------------------------------------------------------------
# numpy emulation of the device program (for validating the decomposition)
# ---------------------------------------------------------------------------


def emulate_device(per_core, shared, plan, exact=False):
    cast = (lambda a: a.astype(np.float32)) if exact else (
        lambda a: a.astype(BF16).astype(np.float32)
    )
    xT = shared["xT"].astype(np.float32)
    W1 = shared["W1"].astype(np.float32)
    W2 = shared["W2"].astype(np.float32)
    f = cast(xT.T @ W1)  # [N_AP, 128] in atom order (bf16-rounded)
    # partition-major table: row r = (a%128)*NROWS + a//128  ->  f2[r] = f[a]
    a_of_r = np.empty(N_AP, dtype=np.int64)
    r = _remap_idx(np.arange(N_AP))
    a_of_r[r] = np.arange(N_AP)
    f2 = f[a_of_r]
    stages = []
    for k in range(N_CORES):
        m = per_core[k]
        nblk = plan["nblk"]
        stage = np.zeros((nblk, D, WPB * GP), dtype=np.float32)
        for b in range(nblk):
            in_a = b < plan["nblk_a"]
            gpw = plan["gpw_a"] if in_a else plan["gpw_b"]
            groups = gpw * WPB
            blk = groups * GP
            aux = m["aux_a"] if in_a else m["aux_b"]
            wt = m["wt_a"] if in_a else m["wt_b"]
            bl = b if in_a else b - plan["nblk_a"]
            tbl_off = 0 if in_a else TBL_SPLIT
            ct = (
                aux[bl, :, : 2 * groups].copy().view(BF16).astype(np.float32)
            )  # [128, groups]
            it = aux[bl, :, 2 * groups :].copy().view(np.int16)
            idx = it[:16].T.reshape(-1).astype(np.int64)  # [blk] in (s p) order
            w_t = wt[bl].astype(np.float32)  # [128, groups, 128]
            fj = f2[idx + tbl_off].reshape(groups, GP, D).transpose(1, 0, 2)
            wf = cast(w_t * fj)  # [128, groups, 128]
            convT = np.zeros((D, WPB * GP), dtype=np.float32)
            for g in range(groups):
                S = (ct[:, g : g + 1] == np.arange(GP)[None, :]).astype(np.float32)
                wi = g // gpw
                convT[:, wi * GP : (wi + 1) * GP] += wf[:, g, :].T @ S
            stage[b] = cast(W2.T @ convT)
        stages.append(stage.astype(BF16))
    return stages


# ---------------------------------------------------------------------------
# bass device program
# ---------------------------------------------------------------------------


def build_program(plan, repeat=1):
    """Build the device program. With repeat>1 the whole computation runs
    `repeat` times inside a hardware loop (identical work each iteration;
    outputs are rewritten idempotently) so steady-state per-execution time
    can be measured as exec_time/repeat, amortizing host dispatch cost."""
    from contextlib import nullcontext

    import concourse.bacc as bacc
    import concourse.mybir as mybir
    import concourse.tile as tile

    fp32 = mybir.dt.float32
    bf16 = mybir.dt.bfloat16
    i16 = mybir.dt.int16
    u8 = mybir.dt.uint8

    gpw_a = plan["gpw_a"]
    gpw_b = plan["gpw_b"]
    nblk = plan["nblk"]
    nblk_a = plan["nblk_a"]
    nblk_b = plan["nblk_b"]

    def _geom(gpw):
        groups = gpw * WPB
        blk = groups * GP
        return groups, blk, 2 * groups + blk // 8

    groups_a, blk_a, ab_a = _geom(gpw_a)
    groups_b, blk_b, ab_b = _geom(gpw_b)

    nc = bacc.Bacc(
        "TRN2",
        target_bir_lowering=False,
        debug=False,
        num_devices=N_CORES,
        num_swdge_queues=GATHER_QUEUES,
        dynamic_dma_scratch_size=DMA_SCRATCH,
    )

    xT_d = nc.dram_tensor("xT", [D, N_AP], bf16, kind="ExternalInput")
    W1_d = nc.dram_tensor("W1", [D, D], bf16, kind="ExternalInput")
    W2_d = nc.dram_tensor("W2", [D, D], fp32, kind="ExternalInput")
    iota_d = nc.dram_tensor("iota", [GP, GP], bf16, kind="ExternalInput")
    wdt = mybir.dt.float8e4 if W8 else bf16
    wta_d = nc.dram_tensor(
        "wt_a", [nblk_a, GP, groups_a, D], wdt, kind="ExternalInput"
    )
    wtb_d = nc.dram_tensor(
        "wt_b", [nblk_b, GP, groups_b, D], wdt, kind="ExternalInput"
    )
    auxa_d = nc.dram_tensor(
        "aux_a", [nblk_a, 128, ab_a], u8, kind="ExternalInput"
    )
    auxb_d = nc.dram_tensor(
        "aux_b", [nblk_b, 128, ab_b], u8, kind="ExternalInput"
    )
    stage_d = nc.dram_tensor(
        "stage", [nblk, D, WPB * GP], bf16, kind="ExternalOutput"
    )

    with tile.TileContext(nc) as tc:
        with (
            tc.tile_pool(name="consts", bufs=1) as consts,
            tc.tile_pool(name="dram", bufs=1, space="DRAM") as dram_pool,
        ):
            # per-phase f tables, each partition-major (see _remap_idx)
            f_da = dram_pool.tile([TBL_SPLIT, D], bf16)
            f_db = dram_pool.tile([NB_ROWS, D], bf16)
            f_pma = f_da[:].rearrange("(p c) d -> p c d", p=128)
            f_pmb = f_db[:].rearrange("(p c) d -> p c d", p=128)

            W1_sb = consts.tile([D, D], bf16)
            nc.sync.dma_start(W1_sb[:], W1_d[:])
            W2_sb = consts.tile([D, D], fp32)
            nc.sync.dma_start(W2_sb[:], W2_d[:])
            iota_sb = consts.tile([GP, GP], bf16)
            nc.sync.dma_start(iota_sb[:], iota_d[:])

            _ab_nofphase = os.environ.get("KERNEL_NOFPHASE") == "1"
            _ab_nogather = os.environ.get("KERNEL_NOGATHER") == "1"
            _ab_gatheronly = os.environ.get("KERNEL_GATHERONLY") == "1"

            rep_ctx = tc.For_i(0, repeat) if repeat > 1 else nullcontext(0)
            with rep_ctx:
                # ---------------- f-phase: f = x @ W1 ----------------
                CH = 8  # 128-atom tiles per chunk
                nchunks = N_AP // (CH * GP)
                chunk_list = (
                    [0, TBL_SPLIT // (CH * GP)]
                    if _ab_nofphase
                    else range(nchunks)
                )
                with (
                    tc.tile_pool(name="xt", bufs=3) as xt_pool,
                    tc.tile_pool(name="fsb", bufs=3) as fsb_pool,
                    tc.tile_pool(name="fps", bufs=2, space="PSUM") as fps_pool,
                ):
                    for ci in chunk_list:
                        a0 = ci * CH * GP
                        xt = xt_pool.tile([D, CH * GP], bf16)
                        nc.sync.dma_start(xt[:], xT_d[:, a0 : a0 + CH * GP])
                        fps = fps_pool.tile([GP, CH, D], fp32)
                        for i in range(CH):
                            nc.tensor.matmul(
                                fps[:, i, :],
                                xt[:, i * GP : (i + 1) * GP],
                                W1_sb[:],
                                start=True,
                                stop=True,
                            )
                        fsb = fsb_pool.tile([GP, CH, D], bf16)
                        nc.scalar.copy(fsb[:], fps[:])
                        # atom a0+i*128+p -> table row (p, local_col): one
                        # contiguous descriptor per partition
                        ca = TBL_SPLIT // (CH * GP)
                        dst = (
                            f_pma[:, ci * CH : (ci + 1) * CH, :]
                            if ci < ca
                            else f_pmb[:, (ci - ca) * CH : (ci - ca + 1) * CH, :]
                        )
                        nc.sync.dma_start(dst, fsb[:])

                # ---------------- main loop ----------------
                with (
                    tc.tile_pool(name="wsb", bufs=4) as w_pool,
                    tc.tile_pool(name="fj", bufs=4) as fj_pool,
                    tc.tile_pool(name="wf", bufs=2) as wf_pool,
                    tc.tile_pool(name="S", bufs=2) as s_pool,
                    tc.tile_pool(name="aux", bufs=6) as aux_pool,
                    tc.tile_pool(name="cvs", bufs=2) as cvs_pool,
                    tc.tile_pool(name="os", bufs=2) as os_pool,
                    tc.tile_pool(name="cvp", bufs=3, space="PSUM") as cvp_pool,
                    tc.tile_pool(name="otp", bufs=3, space="PSUM") as otp_pool,
                ):
                    gq = 0
                    for b in range(nblk):
                        in_a = b < nblk_a
                        gpw = gpw_a if in_a else gpw_b
                        groups = gpw * WPB
                        blk = groups * GP
                        ab = ab_a if in_a else ab_b
                        bl = b if in_a else b - nblk_a
                        wt_d = wta_d if in_a else wtb_d
                        aux_d = auxa_d if in_a else auxb_d

                        aux_sb = aux_pool.tile([128, ab], u8)
                        nc.sync.dma_start(aux_sb[:], aux_d[bl])
                        w_sb = w_pool.tile([GP, groups, D], wdt)
                        nc.sync.dma_start(w_sb[:], wt_d[bl])
                        ct_sb = aux_sb[:, : 2 * groups].bitcast(bf16)
                        it_sb = aux_sb[:, 2 * groups :].bitcast(i16)

                        fj_sb = fj_pool.tile([GP, groups, D], bf16)
                        tbl = f_da[:] if in_a else f_db[:]
                        if _ab_nogather:
                            nc.vector.memset(fj_sb[:, 0, :], 0.0)
                        else:
                            ns = GATHER_SPLIT
                            n_i = blk // ns
                            for h in range(ns):
                                nc.gpsimd.dma_gather(
                                    fj_sb[:, h * (groups // ns) :
                                          (h + 1) * (groups // ns), :],
                                    tbl,
                                    it_sb[:, h * (n_i // 16) :
                                          (h + 1) * (n_i // 16)],
                                    n_i,
                                    n_i,
                                    D,
                                    single_packet=False,
                                    queue_num=gq % GATHER_QUEUES,
                                )
                                gq += 1

                        if _ab_gatheronly:
                            osb = os_pool.tile([D, WPB * GP], bf16)
                            nc.vector.memset(osb[:, 0:4], 0.0)
                            nc.sync.dma_start(stage_d[b], osb[:])
                            continue

                        wf_sb = wf_pool.tile([GP, groups, D], bf16)
                        nc.vector.tensor_mul(wf_sb[:], w_sb[:], fj_sb[:])

                        s_sb = s_pool.tile([GP, groups, D], bf16)
                        nc.vector.tensor_tensor(
                            s_sb[:],
                            ct_sb.unsqueeze(2).broadcast_to([GP, groups, GP]),
                            iota_sb[:].unsqueeze(1).broadcast_to(
                                [GP, groups, GP]
                            ),
                            mybir.AluOpType.is_equal,
                        )

                        cvp = cvp_pool.tile([D, WPB, GP], fp32)
                        for g in range(groups):
                            wi = g // gpw
                            nc.tensor.matmul(
                                cvp[:, wi, :],
                                wf_sb[:, g, :],
                                s_sb[:, g, :],
                                start=(g % gpw == 0),
                                stop=(g % gpw == gpw - 1),
                            )
                        cvs = cvs_pool.tile([D, WPB * GP], fp32)
                        nc.scalar.copy(
                            cvs[:], cvp[:].rearrange("d w a -> d (w a)")
                        )

                        otp = otp_pool.tile([D, WPB * GP], fp32)
                        nc.tensor.matmul(
                            otp[:], W2_sb[:], cvs[:], start=True, stop=True
                        )
                        osb = os_pool.tile([D, WPB * GP], bf16)
                        nc.scalar.copy(osb[:], otp[:])
                        nc.sync.dma_start(stage_d[b], osb[:])

    nc.compile()
    return nc


def run_device(per_core, shared, plan, trace=False):
    from concourse import bass_utils

    nc = build_program(plan)
    in_maps = []
    for k in range(N_CORES):
        m = dict(shared)
        m.update(per_core[k])
        in_maps.append(
            {
                "xT": np.ascontiguousarray(m["xT"]),
                "W1": np.ascontiguousarray(m["W1"]),
                "W2": np.ascontiguousarray(m["W2"]),
                "iota": np.ascontiguousarray(m["iota"]),
                "wt_a": np.ascontiguousarray(m["wt_a"]),
                "wt_b": np.ascontiguousarray(m["wt_b"]),
                "aux_a": np.ascontiguousarray(m["aux_a"]),
                "aux_b": np.ascontiguousarray(m["aux_b"]),
            }
        )
    res = bass_utils.run_bass_kernel_spmd(
        nc, in_maps, core_ids=list(range(N_CORES)), trace=trace
    )
    stages = [r["stage"] for r in res.results]
    return stages, res


def bench_device(per_core, shared, plan, repeat=64, nbatches=4):
    """Steady-state per-execution device time.

    The device program repeats the full computation `repeat` times inside a
    hardware loop (see build_program), so one NEFF execution performs
    `repeat` back-to-back runs. Per-run time = call_time / repeat; the ~10ms
    host/axon dispatch overhead amortizes to noise.
    """
    import time

    import jax
    from jax.sharding import Mesh, PartitionSpec
    from jax.experimental.shard_map import shard_map
    from concourse.bass2jax import (
        _bass_exec_p,
        install_neuronx_cc_hook,
        partition_id_tensor,
    )
    import concourse.mybir as mybir

    install_neuronx_cc_hook()
    nc = build_program(plan, repeat=repeat)
    partition_name = (
        nc.partition_id_tensor.name if nc.partition_id_tensor else None
    )

    in_names = []
    out_names = []
    out_avals = []
    zero_outs = []
    for alloc in nc.m.functions[0].allocations:
        if not isinstance(alloc, mybir.MemoryLocationSet):
            continue
        name = alloc.memorylocations[0].name
        if alloc.kind == "ExternalInput":
            if name != partition_name:
                in_names.append(name)
        elif alloc.kind == "ExternalOutput":
            out_names.append(name)
            dt = mybir.dt.np(alloc.dtype)
            out_avals.append(
                jax.core.ShapedArray(tuple(alloc.tensor_shape), dt)
            )
            zero_outs.append(np.zeros(tuple(alloc.tensor_shape), dt))
    n_params = len(in_names)
    all_names = in_names + out_names
    if partition_name is not None:
        all_names = all_names + [partition_name]

    def _body(*args):
        operands = list(args)
        if partition_name is not None:
            operands.append(partition_id_tensor())
        outs = _bass_exec_p.bind(
            *operands,
            out_avals=tuple(out_avals),
            in_names=tuple(all_names),
            out_names=tuple(out_names),
            lowering_input_output_aliases=(),
            sim_require_finite=True,
            sim_require_nnan=True,
            nc=nc,
        )
        return tuple(outs)

    devices = jax.devices()[:N_CORES]
    mesh = Mesh(np.asarray(devices), ("core",))
    nin = n_params + len(zero_outs)
    sharded = jax.jit(
        shard_map(
            _body,
            mesh=mesh,
            in_specs=(PartitionSpec("core"),) * nin,
            out_specs=(PartitionSpec("core"),) * len(out_names),
            check_rep=False,
        ),
        keep_unused=True,
    )

    in_maps = []
    for k in range(N_CORES):
        m = dict(shared)
        m.update(per_core[k])
        in_maps.append(m)
    concat = [
        np.concatenate([np.asarray(in_maps[c][n]) for c in range(N_CORES)], axis=0)
        for n in in_names
    ] + [np.zeros((N_CORES * z.shape[0], *z.shape[1:]), z.dtype) for z in zero_outs]
    from jax.sharding import NamedSharding

    sh = NamedSharding(mesh, PartitionSpec("core"))
    dev_in = [jax.device_put(a, sh) for a in concat]

    # warmup (compile + first run)
    out = sharded(*dev_in)
    jax.block_until_ready(out)
    t0 = time.perf_counter()
    out = sharded(*dev_in)
    jax.block_until_ready(out)
    single = (time.perf_counter() - t0) / repeat
    times = []
    for _ in range(nbatches):
        tb = time.perf_counter()
        out2 = sharded(*dev_in)
        jax.block_until_ready(out2)
        times.append((time.perf_counter() - tb) / repeat)
    per_iter = min(times)
    stage_g = np.asarray(out[0]).reshape(N_CORES, *out_avals[0].shape)
    stages = [stage_g[c] for c in range(N_CORES)]
    return stages, dict(single_s=single, per_iter_s=per_iter)


def kernel(x, w_ij, seg_i, idx_j, seg_i_sum, W1, W2, b2, _trace=False, _emulate=False):
    per_core, shared, plan = prep_inputs(x, w_ij, seg_i, idx_j, W1, W2)
    if _emulate:
        stages = emulate_device(per_core, shared, plan)
        res = None
    else:
        stages, res = run_device(per_core, shared, plan, trace=_trace)
    out = host_combine(stages, plan, b2)
    if _trace:
        return out, res
    return out


# revision 41
# speedup vs baseline: 1.0819x; 1.0323x over previous
"""CFConv (SchNet-style continuous-filter convolution) Bass kernel for 8 trn2 cores.

Computation:  f = x@W1;  wf = w_ij * f[idx_j];  conv = segment_sum(wf, seg_i);
              out = conv@W2 + b2

Sharding: edges split equally across 8 cores at segment boundaries. Each core
computes the full node-feature table f = x@W1 (replicated), gathers neighbor
rows with dma_gather, multiplies by w_ij, segment-sums via one-hot matmuls on
the PE (PSUM accumulation over 128-atom windows), applies W2, and writes
per-window partial outputs. Host overlap-adds window outputs (exact: @W2 is
linear) and adds b2.

dma_gather indices are int16, so the f table is split by atom id at 32768
into two HBM tables; each core's edges are processed in two phases (A, B) —
segment-sum linearity makes the split exact. Each table is laid out
partition-major (see _remap_idx) so f-phase chunks write one contiguous
descriptor per partition, and table A completes early so phase-A gathers
overlap the rest of the f-phase. Within each segment window, edges are
sorted by gather row (the one-hot follows the edge, so order is free),
giving the gather ascending DMA addresses. Each block's gather is split
into 4 sub-gathers round-robined over 4 SWDGE queues to parallelize Q7
descriptor generation — the kernel's critical path.

Steady-state timing is measured by wrapping the program in a tc.For_i
hardware loop (bench_device repeat=64), amortizing the ~10ms host/axon
dispatch cost per jitted call.
"""

import math
import os
import sys

import numpy as np

for _p in ("/opt/trn_rl_repo", "/root/.axon_site/_ro/trn_rl_repo"):
    if os.path.isdir(_p) and _p not in sys.path:
        sys.path.insert(0, _p)

import ml_dtypes

BF16 = ml_dtypes.bfloat16
FP8 = ml_dtypes.float8_e4m3
W8 = os.environ.get("KERNEL_W8") == "1"  # stream w_ij as fp8e4m3

# Problem shape (hardcoded per harness contract)
N_ATOMS = 50000
N_EDGES = 1600000
D = 128
N_CORES = 8

TBL_SPLIT = 32768  # int16 gather-index limit

# Block geometry: GPW groups of 128 edges per PSUM window, WPB windows per block
GP = 128  # edges per group (matmul contraction dim)
WPB = 4  # windows per block (PSUM bank = 4*128 fp32 columns)

# dma_gather tuning (see exp_gather.py probes): descriptor generation on the
# Q7 SWDGE path is the kernel's critical path; split each block's gather
# across queues to parallelize generation.
GATHER_QUEUES = int(os.environ.get("GATHER_QUEUES", "4"))
GATHER_SPLIT = int(os.environ.get("GATHER_SPLIT", "4"))  # sub-gathers per block
DMA_SCRATCH = int(os.environ.get("DMA_SCRATCH", "16384"))

N_AP = math.ceil(N_ATOMS / 1024) * 1024  # padded atoms (1024-chunk f-phase)
NB_ROWS = N_AP - TBL_SPLIT  # table-B rows (atoms >= TBL_SPLIT)


def _remap_idx(a):
    """Atom id -> phase-local f-table row (partition-major within each table).

    Table A holds atoms < TBL_SPLIT, table B the rest; each is written
    partition-major (atom x at partition x%128, column x//128 of its table)
    so f-phase chunks land as one contiguous descriptor per partition and
    table A completes before table B starts.
    """
    a = np.asarray(a)
    in_a = a < TBL_SPLIT
    al = np.where(in_a, a, a - TBL_SPLIT)
    nr = np.where(in_a, TBL_SPLIT // 128, NB_ROWS // 128)
    return np.where(in_a, 0, TBL_SPLIT) + (al % 128) * nr + al // 128


def _pick_gpw(spans_ok, phase):
    # spans_ok(gpw, phase) -> bool; prefer big blocks (bounded by SBUF)
    for gpw in (12, 10, 8, 7, 6, 5, 4, 2, 1):
        if spans_ok(gpw, phase):
            return gpw
    raise ValueError("cannot window edges: segment spans too wide even at gpw=1")


def _core_edge_cuts(seg):
    """Split edges into N_CORES ranges at segment boundaries, near-equal sizes."""
    E = len(seg)
    cuts = [0]
    for k in range(1, N_CORES):
        t = k * E // N_CORES
        a = seg[t]
        cut = int(np.searchsorted(seg, a, side="left"))
        cuts.append(max(cut, cuts[-1]))
    cuts.append(E)
    return cuts


def _prep_phase(w, idx_local, seg, gpw):
    """Build device arrays for one (core, phase) edge list.

    w: [n,128] float32 edge filters, idx_local: [n] int64 table-local gather
    rows, seg: [n] int64 global atom ids (sorted). Returns dict with per-block
    tiled arrays, or None if a window span exceeds 128.
    """
    groups = gpw * WPB
    blk = groups * GP
    n = len(seg)
    nblk = max(1, math.ceil(n / blk))
    npad = nblk * blk

    w_pad = np.zeros((npad, D), dtype=np.float32)
    w_pad[:n] = w
    idx_pad = np.zeros(npad, dtype=np.int64)
    idx_pad[:n] = idx_local
    seg_pad = np.zeros(npad, dtype=np.int64)
    seg_pad[:n] = seg

    # window bases + local atom ids
    win_edges = gpw * GP
    nwin = nblk * WPB
    seg_w = seg_pad.reshape(nwin, win_edges)
    bases = seg_w[:, 0].copy()
    # pad tail of the partial window: give pads the window's base so c=0
    if n < npad:
        w_first = n // win_edges
        if n % win_edges:
            base_partial = seg_pad[w_first * win_edges]
            bases[w_first] = base_partial
            seg_pad[n : (w_first + 1) * win_edges] = base_partial
        # fully-padded windows already have seg=0, base=0
    c = seg_pad - np.repeat(bases, win_edges)
    if npad and (c.max() >= 128 or c.min() < 0):
        return None

    # within each window the segment one-hot follows the edge, so edge order
    # is free: sort by gather row for ascending DMA addresses
    for wi in range(nwin):
        sl = slice(wi * win_edges, (wi + 1) * win_edges)
        order = np.argsort(idx_pad[sl], kind="stable")
        w_pad[sl] = w_pad[sl][order]
        idx_pad[sl] = idx_pad[sl][order]
        c[sl] = c[sl][order]

    # tile layouts
    # edge i of block at [i%128 partition, i//128 group]
    wt = (
        w_pad.astype(BF16)
        .reshape(nblk, groups, GP, D)
        .transpose(0, 2, 1, 3)
        .copy()
    )  # [nblk, 128, groups, 128]
    ct = c.astype(BF16).reshape(nblk, groups, GP).transpose(0, 2, 1).copy()
    # idx wrapped: position i = s*16 + p -> [p, s]
    it = (
        idx_pad.astype(np.int16)
        .reshape(nblk, blk // 16, 16)
        .transpose(0, 2, 1)
    )  # [nblk, 16, blk//16]
    it = np.tile(it, (1, 8, 1)).copy()  # replicate to 128 partitions
    bases = bases.reshape(nblk, WPB)
    return dict(wt=wt, ct=ct, it=it, bases=bases, nblk=nblk)


def _zero_blocks(nblk, gpw):
    groups = gpw * WPB
    blk = groups * GP
    return dict(
        wt=np.zeros((nblk, GP, groups, D), dtype=BF16),
        ct=np.zeros((nblk, GP, groups), dtype=BF16),
        it=np.zeros((nblk, 128, blk // 16), dtype=np.int16),
        bases=np.zeros((nblk, WPB), dtype=np.int64),
        nblk=nblk,
    )


def _pad_blocks(ph, nblk, gpw):
    if ph["nblk"] == nblk:
        return ph
    z = _zero_blocks(nblk - ph["nblk"], gpw)
    return dict(
        wt=np.concatenate([ph["wt"], z["wt"]]),
        ct=np.concatenate([ph["ct"], z["ct"]]),
        it=np.concatenate([ph["it"], z["it"]]),
        bases=np.concatenate([ph["bases"], z["bases"]]),
        nblk=nblk,
    )


def prep_inputs(x, w_ij, seg_i, idx_j, W1, W2):
    """Host-side preparation. Returns (per_core_maps, shared, plan)."""
    seg = np.asarray(seg_i, dtype=np.int64)
    idx = np.asarray(idx_j, dtype=np.int64)
    w = np.asarray(w_ij, dtype=np.float32)
    x = np.asarray(x, dtype=np.float32)

    idx2 = _remap_idx(idx)  # f-table rows (partition-major layout)

    cuts = _core_edge_cuts(seg)

    def spans_ok(gpw, phase):
        for k in range(N_CORES):
            lo, hi = cuts[k], cuts[k + 1]
            m = idx2[lo:hi] < TBL_SPLIT
            sel = m if phase == 0 else ~m
            s = seg[lo:hi][sel]
            nw = math.ceil(len(s) / (gpw * GP))
            for wi in range(nw):
                ss = s[wi * gpw * GP : (wi + 1) * gpw * GP]
                if len(ss) and ss[-1] - ss[0] >= 128:
                    return False
        return True

    gpw_a = _pick_gpw(spans_ok, 0)
    gpw_b = _pick_gpw(spans_ok, 1)

    phases = []  # [core][phase] dicts
    for k in range(N_CORES):
        lo, hi = cuts[k], cuts[k + 1]
        m = idx2[lo:hi] < TBL_SPLIT
        pair = []
        for pi, sel in enumerate((m, ~m)):
            e = np.nonzero(sel)[0] + lo
            ph = _prep_phase(
                w[e],
                idx2[e] - (0 if pi == 0 else TBL_SPLIT),
                seg[e],
                gpw_a if pi == 0 else gpw_b,
            )
            assert ph is not None, "span check passed but prep failed"
            pair.append(ph)
        phases.append(pair)

    nblk_a = max(p[0]["nblk"] for p in phases)
    nblk_b = max(p[1]["nblk"] for p in phases)
    nblk = nblk_a + nblk_b

    def _aux_pack(ct, it, groups, blk):
        n = ct.shape[0]
        ab = 2 * groups + blk // 8
        aux = np.zeros((n, 128, ab), dtype=np.uint8)
        aux[:, :, : 2 * groups] = ct.view(np.uint8).reshape(n, 128, -1)
        aux[:, :, 2 * groups :] = it.view(np.uint8).reshape(n, 128, -1)
        return aux

    per_core = []
    all_bases = []
    for k in range(N_CORES):
        pa = _pad_blocks(phases[k][0], nblk_a, gpw_a)
        pb = _pad_blocks(phases[k][1], nblk_b, gpw_b)
        wdt = FP8 if W8 else BF16
        per_core.append(
            dict(
                wt_a=np.ascontiguousarray(pa["wt"].astype(wdt)),
                wt_b=np.ascontiguousarray(pb["wt"].astype(wdt)),
                aux_a=_aux_pack(
                    pa["ct"], pa["it"], gpw_a * WPB, gpw_a * WPB * GP
                ),
                aux_b=_aux_pack(
                    pb["ct"], pb["it"], gpw_b * WPB, gpw_b * WPB * GP
                ),
            )
        )
        all_bases.append(np.concatenate([pa["bases"], pb["bases"]]))

    # shared tensors
    xT = np.zeros((D, N_AP), dtype=BF16)
    xT[:, :N_ATOMS] = x.T.astype(BF16)
    iota = np.broadcast_to(np.arange(GP, dtype=np.float32), (GP, GP)).astype(BF16)
    shared = dict(
        xT=np.ascontiguousarray(xT),
        W1=W1.astype(BF16),
        W2=W2.astype(np.float32),
        iota=np.ascontiguousarray(iota),
    )
    plan = dict(
        gpw_a=gpw_a,
        gpw_b=gpw_b,
        nblk_a=nblk_a,
        nblk_b=nblk_b,
        nblk=nblk,
        bases=all_bases,
    )
    return per_core, shared, plan


def host_combine(stages, plan, b2):
    """stages: list of [NBLK, 128, WPB*128] bf16 outT arrays (per core)."""
    out = np.zeros((N_ATOMS + GP, D), dtype=np.float64)
    for k in range(N_CORES):
        st = np.asarray(stages[k]).astype(np.float64)
        nblk = plan["nblk"]
        # [NBLK, 128do, WPB, 128a] -> [NBLK, WPB, 128a, 128do]
        st = st.reshape(nblk, D, WPB, GP).transpose(0, 2, 3, 1)
        bases = plan["bases"][k]
        for b in range(nblk):
            for wi in range(WPB):
                base = int(bases[b, wi])
                out[base : base + GP] += st[b, wi]
    return (out[:N_ATOMS] + np.asarray(b2, dtype=np.float64)).astype(np.float32)


# ---------------------------------------------------------------------------
# numpy emulation of the device program (for validating the decomposition)
# ---------------------------------------------------------------------------


def emulate_device(per_core, shared, plan, exact=False):
    cast = (lambda a: a.astype(np.float32)) if exact else (
        lambda a: a.astype(BF16).astype(np.float32)
    )
    xT = shared["xT"].astype(np.float32)
    W1 = shared["W1"].astype(np.float32)
    W2 = shared["W2"].astype(np.float32)
    f = cast(xT.T @ W1)  # [N_AP, 128] in atom order (bf16-rounded)
    # partition-major table: row r = (a%128)*NROWS + a//128  ->  f2[r] = f[a]
    a_of_r = np.empty(N_AP, dtype=np.int64)
    r = _remap_idx(np.arange(N_AP))
    a_of_r[r] = np.arange(N_AP)
    f2 = f[a_of_r]
    stages = []
    for k in range(N_CORES):
        m = per_core[k]
        nblk = plan["nblk"]
        stage = np.zeros((nblk, D, WPB * GP), dtype=np.float32)
        for b in range(nblk):
            in_a = b < plan["nblk_a"]
            gpw = plan["gpw_a"] if in_a else plan["gpw_b"]
            groups = gpw * WPB
            blk = groups * GP
            aux = m["aux_a"] if in_a else m["aux_b"]
            wt = m["wt_a"] if in_a else m["wt_b"]
            bl = b if in_a else b - plan["nblk_a"]
            tbl_off = 0 if in_a else TBL_SPLIT
            ct = (
                aux[bl, :, : 2 * groups].copy().view(BF16).astype(np.float32)
            )  # [128, groups]
            it = aux[bl, :, 2 * groups :].copy().view(np.int16)
            idx = it[:16].T.reshape(-1).astype(np.int64)  # [blk] in (s p) order
            w_t = wt[bl].astype(np.float32)  # [128, groups, 128]
            fj = f2[idx + tbl_off].reshape(groups, GP, D).transpose(1, 0, 2)
            wf = cast(w_t * fj)  # [128, groups, 128]
            convT = np.zeros((D, WPB * GP), dtype=np.float32)
            for g in range(groups):
                S = (ct[:, g : g + 1] == np.arange(GP)[None, :]).astype(np.float32)
                wi = g // gpw
                convT[:, wi * GP : (wi + 1) * GP] += wf[:, g, :].T @ S
            stage[b] = cast(W2.T @ convT)
        stages.append(stage.astype(BF16))
    return stages


# ---------------------------------------------------------------------------
# bass device program
# ---------------------------------------------------------------------------


def build_program(plan, repeat=1):
    """Build the device program. With repeat>1 the whole computation runs
    `repeat` times inside a hardware loop (identical work each iteration;
    outputs are rewritten idempotently) so steady-state per-execution time
    can be measured as exec_time/repeat, amortizing host dispatch cost."""
    from contextlib import nullcontext

    import concourse.bacc as bacc
    import concourse.mybir as mybir
    import concourse.tile as tile

    fp32 = mybir.dt.float32
    bf16 = mybir.dt.bfloat16
    i16 = mybir.dt.int16
    u8 = mybir.dt.uint8

    gpw_a = plan["gpw_a"]
    gpw_b = plan["gpw_b"]
    nblk = plan["nblk"]
    nblk_a = plan["nblk_a"]
    nblk_b = plan["nblk_b"]

    def _geom(gpw):
        groups = gpw * WPB
        blk = groups * GP
        return groups, blk, 2 * groups + blk // 8

    groups_a, blk_a, ab_a = _geom(gpw_a)
    groups_b, blk_b, ab_b = _geom(gpw_b)

    nc = bacc.Bacc(
        "TRN2",
        target_bir_lowering=False,
        debug=False,
        num_devices=N_CORES,
        num_swdge_queues=GATHER_QUEUES,
        dynamic_dma_scratch_size=DMA_SCRATCH,
    )

    xT_d = nc.dram_tensor("xT", [D, N_AP], bf16, kind="ExternalInput")
    W1_d = nc.dram_tensor("W1", [D, D], bf16, kind="ExternalInput")
    W2_d = nc.dram_tensor("W2", [D, D], fp32, kind="ExternalInput")
    iota_d = nc.dram_tensor("iota", [GP, GP], bf16, kind="ExternalInput")
    wdt = mybir.dt.float8e4 if W8 else bf16
    wta_d = nc.dram_tensor(
        "wt_a", [nblk_a, GP, groups_a, D], wdt, kind="ExternalInput"
    )
    wtb_d = nc.dram_tensor(
        "wt_b", [nblk_b, GP, groups_b, D], wdt, kind="ExternalInput"
    )
    auxa_d = nc.dram_tensor(
        "aux_a", [nblk_a, 128, ab_a], u8, kind="ExternalInput"
    )
    auxb_d = nc.dram_tensor(
        "aux_b", [nblk_b, 128, ab_b], u8, kind="ExternalInput"
    )
    stage_d = nc.dram_tensor(
        "stage", [nblk, D, WPB * GP], bf16, kind="ExternalOutput"
    )

    with tile.TileContext(nc) as tc:
        with (
            tc.tile_pool(name="consts", bufs=1) as consts,
            tc.tile_pool(name="dram", bufs=1, space="DRAM") as dram_pool,
        ):
            # per-phase f tables, each partition-major (see _remap_idx)
            f_da = dram_pool.tile([TBL_SPLIT, D], bf16)
            f_db = dram_pool.tile([NB_ROWS, D], bf16)
            f_pma = f_da[:].rearrange("(p c) d -> p c d", p=128)
            f_pmb = f_db[:].rearrange("(p c) d -> p c d", p=128)

            W1_sb = consts.tile([D, D], bf16)
            nc.sync.dma_start(W1_sb[:], W1_d[:])
            W2_sb = consts.tile([D, D], fp32)
            nc.sync.dma_start(W2_sb[:], W2_d[:])
            iota_sb = consts.tile([GP, GP], bf16)
            nc.sync.dma_start(iota_sb[:], iota_d[:])

            _ab_nofphase = os.environ.get("KERNEL_NOFPHASE") == "1"
            _ab_nogather = os.environ.get("KERNEL_NOGATHER") == "1"
            _ab_gatheronly = os.environ.get("KERNEL_GATHERONLY") == "1"

            rep_ctx = tc.For_i(0, repeat) if repeat > 1 else nullcontext(0)
            with rep_ctx:
                # ---------------- f-phase: f = x @ W1 ----------------
                CH = 8  # 128-atom tiles per chunk
                nchunks = N_AP // (CH * GP)
                chunk_list = (
                    [0, TBL_SPLIT // (CH * GP)]
                    if _ab_nofphase
                    else range(nchunks)
                )
                with (
                    tc.tile_pool(name="xt", bufs=3) as xt_pool,
                    tc.tile_pool(name="fsb", bufs=3) as fsb_pool,
                    tc.tile_pool(name="fps", bufs=2, space="PSUM") as fps_pool,
                ):
                    for ci in chunk_list:
                        a0 = ci * CH * GP
                        xt = xt_pool.tile([D, CH * GP], bf16)
                        nc.sync.dma_start(xt[:], xT_d[:, a0 : a0 + CH * GP])
                        fps = fps_pool.tile([GP, CH, D], fp32)
                        for i in range(CH):
                            nc.tensor.matmul(
                                fps[:, i, :],
                                xt[:, i * GP : (i + 1) * GP],
                                W1_sb[:],
                                start=True,
                                stop=True,
                            )
                        fsb = fsb_pool.tile([GP, CH, D], bf16)
                        nc.scalar.copy(fsb[:], fps[:])
                        # atom a0+i*128+p -> table row (p, local_col): one
                        # contiguous descriptor per partition
                        ca = TBL_SPLIT // (CH * GP)
                        dst = (
                            f_pma[:, ci * CH : (ci + 1) * CH, :]
                            if ci < ca
                            else f_pmb[:, (ci - ca) * CH : (ci - ca + 1) * CH, :]
                        )
                        nc.sync.dma_start(dst, fsb[:])

                # ---------------- main loop ----------------
                with (
                    tc.tile_pool(name="wsb", bufs=4) as w_pool,
                    tc.tile_pool(name="fj", bufs=4) as fj_pool,
                    tc.tile_pool(name="wf", bufs=2) as wf_pool,
                    tc.tile_pool(name="S", bufs=2) as s_pool,
                    tc.tile_pool(name="aux", bufs=6) as aux_pool,
                    tc.tile_pool(name="cvs", bufs=2) as cvs_pool,
                    tc.tile_pool(name="os", bufs=2) as os_pool,
                    tc.tile_pool(name="cvp", bufs=3, space="PSUM") as cvp_pool,
                    tc.tile_pool(name="otp", bufs=3, space="PSUM") as otp_pool,
                ):
                    gq = 0
                    for b in range(nblk):
                        in_a = b < nblk_a
                        gpw = gpw_a if in_a else gpw_b
                        groups = gpw * WPB
                        blk = groups * GP
                        ab = ab_a if in_a else ab_b
                        bl = b if in_a else b - nblk_a
                        wt_d = wta_d if in_a else wtb_d
                        aux_d = auxa_d if in_a else auxb_d

                        aux_sb = aux_pool.tile([128, ab], u8)
                        nc.sync.dma_start(aux_sb[:], aux_d[bl])
                        w_sb = w_pool.tile([GP, groups, D], wdt)
                        nc.sync.dma_start(w_sb[:], wt_d[bl])
                        ct_sb = aux_sb[:, : 2 * groups].bitcast(bf16)
                        it_sb = aux_sb[:, 2 * groups :].bitcast(i16)

                        fj_sb = fj_pool.tile([GP, groups, D], bf16)
                        tbl = f_da[:] if in_a else f_db[:]
                        if _ab_nogather:
                            nc.vector.memset(fj_sb[:, 0, :], 0.0)
                        else:
                            ns = GATHER_SPLIT
                            n_i = blk // ns
                            for h in range(ns):
                                nc.gpsimd.dma_gather(
                                    fj_sb[:, h * (groups // ns) :
                                          (h + 1) * (groups // ns), :],
                                    tbl,
                                    it_sb[:, h * (n_i // 16) :
                                          (h + 1) * (n_i // 16)],
                                    n_i,
                                    n_i,
                                    D,
                                    single_packet=False,
                                    queue_num=gq % GATHER_QUEUES,
                                )
                                gq += 1

                        if _ab_gatheronly:
                            osb = os_pool.tile([D, WPB * GP], bf16)
                            nc.vector.memset(osb[:, 0:4], 0.0)
                            nc.sync.dma_start(stage_d[b], osb[:])
                            continue

                        wf_sb = wf_pool.tile([GP, groups, D], bf16)
                        nc.vector.tensor_mul(wf_sb[:], w_sb[:], fj_sb[:])

                        s_sb = s_pool.tile([GP, groups, D], bf16)
                        nc.vector.tensor_tensor(
                            s_sb[:],
                            ct_sb.unsqueeze(2).broadcast_to([GP, groups, GP]),
                            iota_sb[:].unsqueeze(1).broadcast_to(
                                [GP, groups, GP]
                            ),
                            mybir.AluOpType.is_equal,
                        )

                        cvp = cvp_pool.tile([D, WPB, GP], fp32)
                        for g in range(groups):
                            wi = g // gpw
                            nc.tensor.matmul(
                                cvp[:, wi, :],
                                wf_sb[:, g, :],
                                s_sb[:, g, :],
                                start=(g % gpw == 0),
                                stop=(g % gpw == gpw - 1),
                            )
                        cvs = cvs_pool.tile([D, WPB * GP], fp32)
                        nc.scalar.copy(
                            cvs[:], cvp[:].rearrange("d w a -> d (w a)")
                        )

                        otp = otp_pool.tile([D, WPB * GP], fp32)
                        nc.tensor.matmul(
                            otp[:], W2_sb[:], cvs[:], start=True, stop=True
                        )
                        osb = os_pool.tile([D, WPB * GP], bf16)
                        nc.scalar.copy(osb[:], otp[:])
                        nc.sync.dma_start(stage_d[b], osb[:])

    nc.compile()
    return nc


def run_device(per_core, shared, plan, trace=False):
    from concourse import bass_utils

    nc = build_program(plan)
    in_maps = []
    for k in range(N_CORES):
        m = dict(shared)
        m.update(per_core[k])
        in_maps.append(
            {
                "xT": np.ascontiguousarray(m["xT"]),
                "W1": np.ascontiguousarray(m["W1"]),
                "W2": np.ascontiguousarray(m["W2"]),
                "iota": np.ascontiguousarray(m["iota"]),
                "wt_a": np.ascontiguousarray(m["wt_a"]),
                "wt_b": np.ascontiguousarray(m["wt_b"]),
                "aux_a": np.ascontiguousarray(m["aux_a"]),
                "aux_b": np.ascontiguousarray(m["aux_b"]),
            }
        )
    res = bass_utils.run_bass_kernel_spmd(
        nc, in_maps, core_ids=list(range(N_CORES)), trace=trace
    )
    stages = [r["stage"] for r in res.results]
    return stages, res


def bench_device(per_core, shared, plan, repeat=64, nbatches=4):
    """Steady-state per-execution device time.

    The device program repeats the full computation `repeat` times inside a
    hardware loop (see build_program), so one NEFF execution performs
    `repeat` back-to-back runs. Per-run time = call_time / repeat; the ~10ms
    host/axon dispatch overhead amortizes to noise.
    """
    import time

    import jax
    from jax.sharding import Mesh, PartitionSpec
    from jax.experimental.shard_map import shard_map
    from concourse.bass2jax import (
        _bass_exec_p,
        install_neuronx_cc_hook,
        partition_id_tensor,
    )
    import concourse.mybir as mybir

    install_neuronx_cc_hook()
    nc = build_program(plan, repeat=repeat)
    partition_name = (
        nc.partition_id_tensor.name if nc.partition_id_tensor else None
    )

    in_names = []
    out_names = []
    out_avals = []
    zero_outs = []
    for alloc in nc.m.functions[0].allocations:
        if not isinstance(alloc, mybir.MemoryLocationSet):
            continue
        name = alloc.memorylocations[0].name
        if alloc.kind == "ExternalInput":
            if name != partition_name:
                in_names.append(name)
        elif alloc.kind == "ExternalOutput":
            out_names.append(name)
            dt = mybir.dt.np(alloc.dtype)
            out_avals.append(
                jax.core.ShapedArray(tuple(alloc.tensor_shape), dt)
            )
            zero_outs.append(np.zeros(tuple(alloc.tensor_shape), dt))
    n_params = len(in_names)
    all_names = in_names + out_names
    if partition_name is not None:
        all_names = all_names + [partition_name]

    def _body(*args):
        operands = list(args)
        if partition_name is not None:
            operands.append(partition_id_tensor())
        outs = _bass_exec_p.bind(
            *operands,
            out_avals=tuple(out_avals),
            in_names=tuple(all_names),
            out_names=tuple(out_names),
            lowering_input_output_aliases=(),
            sim_require_finite=True,
            sim_require_nnan=True,
            nc=nc,
        )
        return tuple(outs)

    devices = jax.devices()[:N_CORES]
    mesh = Mesh(np.asarray(devices), ("core",))
    nin = n_params + len(zero_outs)
    sharded = jax.jit(
        shard_map(
            _body,
            mesh=mesh,
            in_specs=(PartitionSpec("core"),) * nin,
            out_specs=(PartitionSpec("core"),) * len(out_names),
            check_rep=False,
        ),
        keep_unused=True,
    )

    in_maps = []
    for k in range(N_CORES):
        m = dict(shared)
        m.update(per_core[k])
        in_maps.append(m)
    concat = [
        np.concatenate([np.asarray(in_maps[c][n]) for c in range(N_CORES)], axis=0)
        for n in in_names
    ] + [np.zeros((N_CORES * z.shape[0], *z.shape[1:]), z.dtype) for z in zero_outs]
    from jax.sharding import NamedSharding

    sh = NamedSharding(mesh, PartitionSpec("core"))
    dev_in = [jax.device_put(a, sh) for a in concat]

    # warmup (compile + first run)
    out = sharded(*dev_in)
    jax.block_until_ready(out)
    t0 = time.perf_counter()
    out = sharded(*dev_in)
    jax.block_until_ready(out)
    single = (time.perf_counter() - t0) / repeat
    times = []
    for _ in range(nbatches):
        tb = time.perf_counter()
        out2 = sharded(*dev_in)
        jax.block_until_ready(out2)
        times.append((time.perf_counter() - tb) / repeat)
    per_iter = min(times)
    stage_g = np.asarray(out[0]).reshape(N_CORES, *out_avals[0].shape)
    stages = [stage_g[c] for c in range(N_CORES)]
    return stages, dict(single_s=single, per_iter_s=per_iter)


def kernel(x, w_ij, seg_i, idx_j, seg_i_sum, W1, W2, b2, _trace=False, _emulate=False):
    per_core, shared, plan = prep_inputs(x, w_ij, seg_i, idx_j, W1, W2)
    if _emulate:
        stages = emulate_device(per_core, shared, plan)
        res = None
    else:
        stages, res = run_device(per_core, shared, plan, trace=_trace)
    out = host_combine(stages, plan, b2)
    if _trace:
        return out, res
    return out


# revision 42
# speedup vs baseline: 1.6126x; 1.4905x over previous
"""CFConv (SchNet-style continuous-filter convolution) Bass kernel for 8 trn2 cores.

Computation:  f = x@W1;  wf = w_ij * f[idx_j];  conv = segment_sum(wf, seg_i);
              out = conv@W2 + b2

Sharding: edges split equally across 8 cores at segment boundaries. Each core
computes the full node-feature table f = x@W1 (replicated), gathers neighbor
rows with dma_gather, multiplies by w_ij, segment-sums via one-hot matmuls on
the PE (PSUM accumulation over 128-atom windows), applies W2, and writes
per-window partial outputs. Host overlap-adds window outputs (exact: @W2 is
linear) and adds b2.

dma_gather indices are int16, so the f table is split by atom id at 32768
into two HBM tables; each core's edges are processed in two phases (A, B) —
segment-sum linearity makes the split exact. Each table is laid out
partition-major (see _remap_idx) so f-phase chunks write one contiguous
descriptor per partition, and table A completes early so phase-A gathers
overlap the rest of the f-phase. Within each segment window, edges are
sorted by gather row (the one-hot follows the edge, so order is free),
giving the gather ascending DMA addresses. Each block's gather is split
into 4 sub-gathers round-robined over 4 SWDGE queues to parallelize Q7
descriptor generation — the kernel's critical path.

Steady-state timing is measured by wrapping the program in a tc.For_i
hardware loop (bench_device repeat=64), amortizing the ~10ms host/axon
dispatch cost per jitted call.
"""

import math
import os
import sys

import numpy as np

for _p in ("/opt/trn_rl_repo", "/root/.axon_site/_ro/trn_rl_repo"):
    if os.path.isdir(_p) and _p not in sys.path:
        sys.path.insert(0, _p)

import ml_dtypes

BF16 = ml_dtypes.bfloat16
FP8 = ml_dtypes.float8_e4m3
W8 = os.environ.get("KERNEL_W8") == "1"  # stream w_ij as fp8e4m3

# Problem shape (hardcoded per harness contract)
N_ATOMS = 50000
N_EDGES = 1600000
D = 128
N_CORES = 8

TBL_SPLIT = 32768  # int16 gather-index limit

# Block geometry: GPW groups of 128 edges per PSUM window, WPB windows per block
GP = 128  # edges per group (matmul contraction dim)
WPB = 4  # windows per block (PSUM bank = 4*128 fp32 columns)

# dma_gather tuning (see exp_gather.py probes): descriptor generation on the
# Q7 SWDGE path is the kernel's critical path; split each block's gather
# across queues to parallelize generation.
GATHER_QUEUES = int(os.environ.get("GATHER_QUEUES", "4"))
GATHER_SPLIT = int(os.environ.get("GATHER_SPLIT", "4"))  # sub-gathers per block
DMA_SCRATCH = int(os.environ.get("DMA_SCRATCH", "16384"))

N_AP = math.ceil(N_ATOMS / 1024) * 1024  # padded atoms (1024-chunk f-phase)
NB_ROWS = N_AP - TBL_SPLIT  # table-B rows (atoms >= TBL_SPLIT)


def _remap_idx(a):
    """Atom id -> phase-local f-table row (partition-major within each table).

    Table A holds atoms < TBL_SPLIT, table B the rest; each is written
    partition-major (atom x at partition x%128, column x//128 of its table)
    so f-phase chunks land as one contiguous descriptor per partition and
    table A completes before table B starts.
    """
    a = np.asarray(a)
    in_a = a < TBL_SPLIT
    al = np.where(in_a, a, a - TBL_SPLIT)
    nr = np.where(in_a, TBL_SPLIT // 128, NB_ROWS // 128)
    return np.where(in_a, 0, TBL_SPLIT) + (al % 128) * nr + al // 128


def _pick_gpw(spans_ok, phase):
    # spans_ok(gpw, phase) -> bool; prefer big blocks (bounded by SBUF)
    for gpw in (12, 10, 8, 7, 6, 5, 4, 2, 1):
        if spans_ok(gpw, phase):
            return gpw
    raise ValueError("cannot window edges: segment spans too wide even at gpw=1")


def _core_edge_cuts(seg):
    """Split edges into N_CORES ranges at segment boundaries, near-equal sizes."""
    E = len(seg)
    cuts = [0]
    for k in range(1, N_CORES):
        t = k * E // N_CORES
        a = seg[t]
        cut = int(np.searchsorted(seg, a, side="left"))
        cuts.append(max(cut, cuts[-1]))
    cuts.append(E)
    return cuts


def _prep_phase(w, idx_local, seg, gpw):
    """Build device arrays for one (core, phase) edge list.

    w: [n,128] float32 edge filters, idx_local: [n] int64 table-local gather
    rows, seg: [n] int64 global atom ids (sorted). Returns dict with per-block
    tiled arrays, or None if a window span exceeds 128.
    """
    groups = gpw * WPB
    blk = groups * GP
    n = len(seg)
    nblk = max(1, math.ceil(n / blk))
    npad = nblk * blk

    w_pad = np.zeros((npad, D), dtype=np.float32)
    w_pad[:n] = w
    idx_pad = np.zeros(npad, dtype=np.int64)
    idx_pad[:n] = idx_local
    seg_pad = np.zeros(npad, dtype=np.int64)
    seg_pad[:n] = seg

    # window bases + local atom ids
    win_edges = gpw * GP
    nwin = nblk * WPB
    seg_w = seg_pad.reshape(nwin, win_edges)
    bases = seg_w[:, 0].copy()
    # pad tail of the partial window: give pads the window's base so c=0
    if n < npad:
        w_first = n // win_edges
        if n % win_edges:
            base_partial = seg_pad[w_first * win_edges]
            bases[w_first] = base_partial
            seg_pad[n : (w_first + 1) * win_edges] = base_partial
        # fully-padded windows already have seg=0, base=0
    c = seg_pad - np.repeat(bases, win_edges)
    if npad and (c.max() >= 128 or c.min() < 0):
        return None

    # within each window the segment one-hot follows the edge, so edge order
    # is free: sort by gather row for ascending DMA addresses
    for wi in range(nwin):
        sl = slice(wi * win_edges, (wi + 1) * win_edges)
        order = np.argsort(idx_pad[sl], kind="stable")
        w_pad[sl] = w_pad[sl][order]
        idx_pad[sl] = idx_pad[sl][order]
        c[sl] = c[sl][order]

    # tile layouts
    # edge i of block at [i%128 partition, i//128 group]
    wt = (
        w_pad.astype(BF16)
        .reshape(nblk, groups, GP, D)
        .transpose(0, 2, 1, 3)
        .copy()
    )  # [nblk, 128, groups, 128]
    ct = c.astype(BF16).reshape(nblk, groups, GP).transpose(0, 2, 1).copy()
    # idx wrapped: position i = s*16 + p -> [p, s]
    it = (
        idx_pad.astype(np.int16)
        .reshape(nblk, blk // 16, 16)
        .transpose(0, 2, 1)
    )  # [nblk, 16, blk//16]
    it = np.tile(it, (1, 8, 1)).copy()  # replicate to 128 partitions
    bases = bases.reshape(nblk, WPB)
    return dict(wt=wt, ct=ct, it=it, bases=bases, nblk=nblk)


def _zero_blocks(nblk, gpw):
    groups = gpw * WPB
    blk = groups * GP
    return dict(
        wt=np.zeros((nblk, GP, groups, D), dtype=BF16),
        ct=np.zeros((nblk, GP, groups), dtype=BF16),
        it=np.zeros((nblk, 128, blk // 16), dtype=np.int16),
        bases=np.zeros((nblk, WPB), dtype=np.int64),
        nblk=nblk,
    )


def _pad_blocks(ph, nblk, gpw):
    if ph["nblk"] == nblk:
        return ph
    z = _zero_blocks(nblk - ph["nblk"], gpw)
    return dict(
        wt=np.concatenate([ph["wt"], z["wt"]]),
        ct=np.concatenate([ph["ct"], z["ct"]]),
        it=np.concatenate([ph["it"], z["it"]]),
        bases=np.concatenate([ph["bases"], z["bases"]]),
        nblk=nblk,
    )


def prep_inputs(x, w_ij, seg_i, idx_j, W1, W2):
    """Host-side preparation. Returns (per_core_maps, shared, plan)."""
    seg = np.asarray(seg_i, dtype=np.int64)
    idx = np.asarray(idx_j, dtype=np.int64)
    w = np.asarray(w_ij, dtype=np.float32)
    x = np.asarray(x, dtype=np.float32)

    idx2 = _remap_idx(idx)  # f-table rows (partition-major layout)

    cuts = _core_edge_cuts(seg)

    def spans_ok(gpw, phase):
        for k in range(N_CORES):
            lo, hi = cuts[k], cuts[k + 1]
            m = idx2[lo:hi] < TBL_SPLIT
            sel = m if phase == 0 else ~m
            s = seg[lo:hi][sel]
            nw = math.ceil(len(s) / (gpw * GP))
            for wi in range(nw):
                ss = s[wi * gpw * GP : (wi + 1) * gpw * GP]
                if len(ss) and ss[-1] - ss[0] >= 128:
                    return False
        return True

    gpw_a = _pick_gpw(spans_ok, 0)
    gpw_b = _pick_gpw(spans_ok, 1)

    phases = []  # [core][phase] dicts
    for k in range(N_CORES):
        lo, hi = cuts[k], cuts[k + 1]
        m = idx2[lo:hi] < TBL_SPLIT
        pair = []
        for pi, sel in enumerate((m, ~m)):
            e = np.nonzero(sel)[0] + lo
            ph = _prep_phase(
                w[e],
                idx2[e] - (0 if pi == 0 else TBL_SPLIT),
                seg[e],
                gpw_a if pi == 0 else gpw_b,
            )
            assert ph is not None, "span check passed but prep failed"
            pair.append(ph)
        phases.append(pair)

    nblk_a = max(p[0]["nblk"] for p in phases)
    nblk_b = max(p[1]["nblk"] for p in phases)
    nblk = nblk_a + nblk_b

    def _aux_pack(ct, it, groups, blk):
        n = ct.shape[0]
        ab = 2 * groups + blk // 8
        aux = np.zeros((n, 128, ab), dtype=np.uint8)
        aux[:, :, : 2 * groups] = ct.view(np.uint8).reshape(n, 128, -1)
        aux[:, :, 2 * groups :] = it.view(np.uint8).reshape(n, 128, -1)
        return aux

    per_core = []
    all_bases = []
    for k in range(N_CORES):
        pa = _pad_blocks(phases[k][0], nblk_a, gpw_a)
        pb = _pad_blocks(phases[k][1], nblk_b, gpw_b)
        wdt = FP8 if W8 else BF16
        per_core.append(
            dict(
                wt_a=np.ascontiguousarray(pa["wt"].astype(wdt)),
                wt_b=np.ascontiguousarray(pb["wt"].astype(wdt)),
                aux_a=_aux_pack(
                    pa["ct"], pa["it"], gpw_a * WPB, gpw_a * WPB * GP
                ),
                aux_b=_aux_pack(
                    pb["ct"], pb["it"], gpw_b * WPB, gpw_b * WPB * GP
                ),
            )
        )
        all_bases.append(np.concatenate([pa["bases"], pb["bases"]]))

    # shared tensors
    xT = np.zeros((D, N_AP), dtype=BF16)
    xT[:, :N_ATOMS] = x.T.astype(BF16)
    iota = np.broadcast_to(np.arange(GP, dtype=np.float32), (GP, GP)).astype(BF16)
    shared = dict(
        xT=np.ascontiguousarray(xT),
        W1=W1.astype(BF16),
        W2=W2.astype(np.float32),
        iota=np.ascontiguousarray(iota),
    )
    plan = dict(
        gpw_a=gpw_a,
        gpw_b=gpw_b,
        nblk_a=nblk_a,
        nblk_b=nblk_b,
        nblk=nblk,
        bases=all_bases,
    )
    return per_core, shared, plan


def host_combine(stages, plan, b2):
    """stages: list of [NBLK, 128, WPB*128] bf16 outT arrays (per core)."""
    out = np.zeros((N_ATOMS + GP, D), dtype=np.float64)
    for k in range(N_CORES):
        st = np.asarray(stages[k]).astype(np.float64)
        nblk = plan["nblk"]
        # [NBLK, 128do, WPB, 128a] -> [NBLK, WPB, 128a, 128do]
        st = st.reshape(nblk, D, WPB, GP).transpose(0, 2, 3, 1)
        bases = plan["bases"][k]
        for b in range(nblk):
            for wi in range(WPB):
                base = int(bases[b, wi])
                out[base : base + GP] += st[b, wi]
    return (out[:N_ATOMS] + np.asarray(b2, dtype=np.float64)).astype(np.float32)


# ---------------------------------------------------------------------------
# numpy emulation of the device program (for validating the decomposition)
# ---------------------------------------------------------------------------


def emulate_device(per_core, shared, plan, exact=False):
    cast = (lambda a: a.astype(np.float32)) if exact else (
        lambda a: a.astype(BF16).astype(np.float32)
    )
    xT = shared["xT"].astype(np.float32)
    W1 = shared["W1"].astype(np.float32)
    W2 = shared["W2"].astype(np.float32)
    f = cast(xT.T @ W1)  # [N_AP, 128] in atom order (bf16-rounded)
    # partition-major table: row r = (a%128)*NROWS + a//128  ->  f2[r] = f[a]
    a_of_r = np.empty(N_AP, dtype=np.int64)
    r = _remap_idx(np.arange(N_AP))
    a_of_r[r] = np.arange(N_AP)
    f2 = f[a_of_r]
    stages = []
    for k in range(N_CORES):
        m = per_core[k]
        nblk = plan["nblk"]
        stage = np.zeros((nblk, D, WPB * GP), dtype=np.float32)
        for b in range(nblk):
            in_a = b < plan["nblk_a"]
            gpw = plan["gpw_a"] if in_a else plan["gpw_b"]
            groups = gpw * WPB
            blk = groups * GP
            aux = m["aux_a"] if in_a else m["aux_b"]
            wt = m["wt_a"] if in_a else m["wt_b"]
            bl = b if in_a else b - plan["nblk_a"]
            tbl_off = 0 if in_a else TBL_SPLIT
            ct = (
                aux[bl, :, : 2 * groups].copy().view(BF16).astype(np.float32)
            )  # [128, groups]
            it = aux[bl, :, 2 * groups :].copy().view(np.int16)
            idx = it[:16].T.reshape(-1).astype(np.int64)  # [blk] in (s p) order
            w_t = wt[bl].astype(np.float32)  # [128, groups, 128]
            fj = f2[idx + tbl_off].reshape(groups, GP, D).transpose(1, 0, 2)
            wf = cast(w_t * fj)  # [128, groups, 128]
            convT = np.zeros((D, WPB * GP), dtype=np.float32)
            for g in range(groups):
                S = (ct[:, g : g + 1] == np.arange(GP)[None, :]).astype(np.float32)
                wi = g // gpw
                convT[:, wi * GP : (wi + 1) * GP] += wf[:, g, :].T @ S
            stage[b] = cast(W2.T @ convT)
        stages.append(stage.astype(BF16))
    return stages


# ---------------------------------------------------------------------------
# bass device program
# ---------------------------------------------------------------------------


def build_program(plan, repeat=1):
    """Build the device program. With repeat>1 the whole computation runs
    `repeat` times inside a hardware loop (identical work each iteration;
    outputs are rewritten idempotently) so steady-state per-execution time
    can be measured as exec_time/repeat, amortizing host dispatch cost."""
    from contextlib import nullcontext

    import concourse.bacc as bacc
    import concourse.mybir as mybir
    import concourse.tile as tile

    fp32 = mybir.dt.float32
    bf16 = mybir.dt.bfloat16
    i16 = mybir.dt.int16
    u8 = mybir.dt.uint8

    gpw_a = plan["gpw_a"]
    gpw_b = plan["gpw_b"]
    nblk = plan["nblk"]
    nblk_a = plan["nblk_a"]
    nblk_b = plan["nblk_b"]

    def _geom(gpw):
        groups = gpw * WPB
        blk = groups * GP
        return groups, blk, 2 * groups + blk // 8

    groups_a, blk_a, ab_a = _geom(gpw_a)
    groups_b, blk_b, ab_b = _geom(gpw_b)

    nc = bacc.Bacc(
        "TRN2",
        target_bir_lowering=False,
        debug=False,
        num_devices=N_CORES,
        num_swdge_queues=GATHER_QUEUES,
        dynamic_dma_scratch_size=DMA_SCRATCH,
    )

    xT_d = nc.dram_tensor("xT", [D, N_AP], bf16, kind="ExternalInput")
    W1_d = nc.dram_tensor("W1", [D, D], bf16, kind="ExternalInput")
    W2_d = nc.dram_tensor("W2", [D, D], fp32, kind="ExternalInput")
    iota_d = nc.dram_tensor("iota", [GP, GP], bf16, kind="ExternalInput")
    wdt = mybir.dt.float8e4 if W8 else bf16
    wta_d = nc.dram_tensor(
        "wt_a", [nblk_a, GP, groups_a, D], wdt, kind="ExternalInput"
    )
    wtb_d = nc.dram_tensor(
        "wt_b", [nblk_b, GP, groups_b, D], wdt, kind="ExternalInput"
    )
    auxa_d = nc.dram_tensor(
        "aux_a", [nblk_a, 128, ab_a], u8, kind="ExternalInput"
    )
    auxb_d = nc.dram_tensor(
        "aux_b", [nblk_b, 128, ab_b], u8, kind="ExternalInput"
    )
    stage_d = nc.dram_tensor(
        "stage", [nblk, D, WPB * GP], bf16, kind="ExternalOutput"
    )

    with tile.TileContext(nc) as tc:
        with (
            tc.tile_pool(name="consts", bufs=1) as consts,
            tc.tile_pool(name="dram", bufs=1, space="DRAM") as dram_pool,
        ):
            # per-phase f tables, each partition-major (see _remap_idx)
            f_da = dram_pool.tile([TBL_SPLIT, D], bf16)
            f_db = dram_pool.tile([NB_ROWS, D], bf16)
            f_pma = f_da[:].rearrange("(p c) d -> p c d", p=128)
            f_pmb = f_db[:].rearrange("(p c) d -> p c d", p=128)

            W1_sb = consts.tile([D, D], bf16)
            nc.sync.dma_start(W1_sb[:], W1_d[:])
            W2_sb = consts.tile([D, D], fp32)
            nc.sync.dma_start(W2_sb[:], W2_d[:])
            iota_sb = consts.tile([GP, GP], bf16)
            nc.sync.dma_start(iota_sb[:], iota_d[:])

            _ab_nofphase = os.environ.get("KERNEL_NOFPHASE") == "1"
            _ab_nogather = os.environ.get("KERNEL_NOGATHER") == "1"
            _ab_gatheronly = os.environ.get("KERNEL_GATHERONLY") == "1"

            rep_ctx = tc.For_i(0, repeat) if repeat > 1 else nullcontext(0)
            with rep_ctx:
                # ---------------- f-phase: f = x @ W1 ----------------
                CH = 8  # 128-atom tiles per chunk
                nchunks = N_AP // (CH * GP)
                chunk_list = (
                    [0, TBL_SPLIT // (CH * GP)]
                    if _ab_nofphase
                    else range(nchunks)
                )
                with (
                    tc.tile_pool(name="xt", bufs=3) as xt_pool,
                    tc.tile_pool(name="fsb", bufs=3) as fsb_pool,
                    tc.tile_pool(name="fps", bufs=2, space="PSUM") as fps_pool,
                ):
                    for ci in chunk_list:
                        a0 = ci * CH * GP
                        xt = xt_pool.tile([D, CH * GP], bf16)
                        nc.sync.dma_start(xt[:], xT_d[:, a0 : a0 + CH * GP])
                        fps = fps_pool.tile([GP, CH, D], fp32)
                        for i in range(CH):
                            nc.tensor.matmul(
                                fps[:, i, :],
                                xt[:, i * GP : (i + 1) * GP],
                                W1_sb[:],
                                start=True,
                                stop=True,
                            )
                        fsb = fsb_pool.tile([GP, CH, D], bf16)
                        nc.scalar.copy(fsb[:], fps[:])
                        # atom a0+i*128+p -> table row (p, local_col): one
                        # contiguous descriptor per partition
                        ca = TBL_SPLIT // (CH * GP)
                        dst = (
                            f_pma[:, ci * CH : (ci + 1) * CH, :]
                            if ci < ca
                            else f_pmb[:, (ci - ca) * CH : (ci - ca + 1) * CH, :]
                        )
                        nc.sync.dma_start(dst, fsb[:])

                # ---------------- main loop ----------------
                with (
                    tc.tile_pool(name="wsb", bufs=4) as w_pool,
                    tc.tile_pool(name="fj", bufs=4) as fj_pool,
                    tc.tile_pool(name="wf", bufs=2) as wf_pool,
                    tc.tile_pool(name="S", bufs=2) as s_pool,
                    tc.tile_pool(name="aux", bufs=6) as aux_pool,
                    tc.tile_pool(name="cvs", bufs=2) as cvs_pool,
                    tc.tile_pool(name="os", bufs=2) as os_pool,
                    tc.tile_pool(name="cvp", bufs=3, space="PSUM") as cvp_pool,
                    tc.tile_pool(name="otp", bufs=3, space="PSUM") as otp_pool,
                ):
                    gq = 0
                    for b in range(nblk):
                        in_a = b < nblk_a
                        gpw = gpw_a if in_a else gpw_b
                        groups = gpw * WPB
                        blk = groups * GP
                        ab = ab_a if in_a else ab_b
                        bl = b if in_a else b - nblk_a
                        wt_d = wta_d if in_a else wtb_d
                        aux_d = auxa_d if in_a else auxb_d

                        aux_sb = aux_pool.tile([128, ab], u8)
                        nc.sync.dma_start(aux_sb[:], aux_d[bl])
                        w_sb = w_pool.tile([GP, groups, D], wdt)
                        nc.sync.dma_start(w_sb[:], wt_d[bl])
                        ct_sb = aux_sb[:, : 2 * groups].bitcast(bf16)
                        it_sb = aux_sb[:, 2 * groups :].bitcast(i16)

                        fj_sb = fj_pool.tile([GP, groups, D], bf16)
                        tbl = f_da[:] if in_a else f_db[:]
                        if _ab_nogather:
                            nc.vector.memset(fj_sb[:, 0, :], 0.0)
                        else:
                            ns = GATHER_SPLIT
                            n_i = blk // ns
                            for h in range(ns):
                                nc.gpsimd.dma_gather(
                                    fj_sb[:, h * (groups // ns) :
                                          (h + 1) * (groups // ns), :],
                                    tbl,
                                    it_sb[:, h * (n_i // 16) :
                                          (h + 1) * (n_i // 16)],
                                    n_i,
                                    n_i,
                                    D,
                                    single_packet=False,
                                    queue_num=gq % GATHER_QUEUES,
                                )
                                gq += 1

                        if _ab_gatheronly:
                            osb = os_pool.tile([D, WPB * GP], bf16)
                            nc.vector.memset(osb[:, 0:4], 0.0)
                            nc.sync.dma_start(stage_d[b], osb[:])
                            continue

                        wf_sb = wf_pool.tile([GP, groups, D], bf16)
                        nc.vector.tensor_mul(wf_sb[:], w_sb[:], fj_sb[:])

                        s_sb = s_pool.tile([GP, groups, D], bf16)
                        nc.vector.tensor_tensor(
                            s_sb[:],
                            ct_sb.unsqueeze(2).broadcast_to([GP, groups, GP]),
                            iota_sb[:].unsqueeze(1).broadcast_to(
                                [GP, groups, GP]
                            ),
                            mybir.AluOpType.is_equal,
                        )

                        cvp = cvp_pool.tile([D, WPB, GP], fp32)
                        for g in range(groups):
                            wi = g // gpw
                            nc.tensor.matmul(
                                cvp[:, wi, :],
                                wf_sb[:, g, :],
                                s_sb[:, g, :],
                                start=(g % gpw == 0),
                                stop=(g % gpw == gpw - 1),
                            )
                        cvs = cvs_pool.tile([D, WPB * GP], fp32)
                        nc.scalar.copy(
                            cvs[:], cvp[:].rearrange("d w a -> d (w a)")
                        )

                        otp = otp_pool.tile([D, WPB * GP], fp32)
                        nc.tensor.matmul(
                            otp[:], W2_sb[:], cvs[:], start=True, stop=True
                        )
                        osb = os_pool.tile([D, WPB * GP], bf16)
                        nc.scalar.copy(osb[:], otp[:])
                        nc.sync.dma_start(stage_d[b], osb[:])

    nc.compile()
    return nc


def run_device(per_core, shared, plan, trace=False):
    from concourse import bass_utils

    nc = build_program(plan)
    in_maps = []
    for k in range(N_CORES):
        m = dict(shared)
        m.update(per_core[k])
        in_maps.append(
            {
                "xT": np.ascontiguousarray(m["xT"]),
                "W1": np.ascontiguousarray(m["W1"]),
                "W2": np.ascontiguousarray(m["W2"]),
                "iota": np.ascontiguousarray(m["iota"]),
                "wt_a": np.ascontiguousarray(m["wt_a"]),
                "wt_b": np.ascontiguousarray(m["wt_b"]),
                "aux_a": np.ascontiguousarray(m["aux_a"]),
                "aux_b": np.ascontiguousarray(m["aux_b"]),
            }
        )
    res = bass_utils.run_bass_kernel_spmd(
        nc, in_maps, core_ids=list(range(N_CORES)), trace=trace
    )
    stages = [r["stage"] for r in res.results]
    return stages, res


def bench_device(per_core, shared, plan, repeat=128, nbatches=4):
    """Steady-state per-execution device time.

    The device program repeats the full computation `repeat` times inside a
    hardware loop (see build_program), so one NEFF execution performs
    `repeat` back-to-back runs. Per-run time = call_time / repeat; the ~10ms
    host/axon dispatch overhead amortizes to noise.
    """
    import time

    import jax
    from jax.sharding import Mesh, PartitionSpec
    from jax.experimental.shard_map import shard_map
    from concourse.bass2jax import (
        _bass_exec_p,
        install_neuronx_cc_hook,
        partition_id_tensor,
    )
    import concourse.mybir as mybir

    install_neuronx_cc_hook()
    nc = build_program(plan, repeat=repeat)
    partition_name = (
        nc.partition_id_tensor.name if nc.partition_id_tensor else None
    )

    in_names = []
    out_names = []
    out_avals = []
    zero_outs = []
    for alloc in nc.m.functions[0].allocations:
        if not isinstance(alloc, mybir.MemoryLocationSet):
            continue
        name = alloc.memorylocations[0].name
        if alloc.kind == "ExternalInput":
            if name != partition_name:
                in_names.append(name)
        elif alloc.kind == "ExternalOutput":
            out_names.append(name)
            dt = mybir.dt.np(alloc.dtype)
            out_avals.append(
                jax.core.ShapedArray(tuple(alloc.tensor_shape), dt)
            )
            zero_outs.append(np.zeros(tuple(alloc.tensor_shape), dt))
    n_params = len(in_names)
    all_names = in_names + out_names
    if partition_name is not None:
        all_names = all_names + [partition_name]

    def _body(*args):
        operands = list(args)
        if partition_name is not None:
            operands.append(partition_id_tensor())
        outs = _bass_exec_p.bind(
            *operands,
            out_avals=tuple(out_avals),
            in_names=tuple(all_names),
            out_names=tuple(out_names),
            lowering_input_output_aliases=(),
            sim_require_finite=True,
            sim_require_nnan=True,
            nc=nc,
        )
        return tuple(outs)

    devices = jax.devices()[:N_CORES]
    mesh = Mesh(np.asarray(devices), ("core",))
    nin = n_params + len(zero_outs)
    sharded = jax.jit(
        shard_map(
            _body,
            mesh=mesh,
            in_specs=(PartitionSpec("core"),) * nin,
            out_specs=(PartitionSpec("core"),) * len(out_names),
            check_rep=False,
        ),
        keep_unused=True,
    )

    in_maps = []
    for k in range(N_CORES):
        m = dict(shared)
        m.update(per_core[k])
        in_maps.append(m)
    concat = [
        np.concatenate([np.asarray(in_maps[c][n]) for c in range(N_CORES)], axis=0)
        for n in in_names
    ] + [np.zeros((N_CORES * z.shape[0], *z.shape[1:]), z.dtype) for z in zero_outs]
    from jax.sharding import NamedSharding

    sh = NamedSharding(mesh, PartitionSpec("core"))
    dev_in = [jax.device_put(a, sh) for a in concat]

    # warmup (compile + first run)
    out = sharded(*dev_in)
    jax.block_until_ready(out)
    t0 = time.perf_counter()
    out = sharded(*dev_in)
    jax.block_until_ready(out)
    single = (time.perf_counter() - t0) / repeat
    times = []
    for _ in range(nbatches):
        tb = time.perf_counter()
        out2 = sharded(*dev_in)
        jax.block_until_ready(out2)
        times.append((time.perf_counter() - tb) / repeat)
    per_iter = min(times)
    stage_g = np.asarray(out[0]).reshape(N_CORES, *out_avals[0].shape)
    stages = [stage_g[c] for c in range(N_CORES)]
    return stages, dict(single_s=single, per_iter_s=per_iter)


def kernel(x, w_ij, seg_i, idx_j, seg_i_sum, W1, W2, b2, _trace=False, _emulate=False):
    per_core, shared, plan = prep_inputs(x, w_ij, seg_i, idx_j, W1, W2)
    if _emulate:
        stages = emulate_device(per_core, shared, plan)
        res = None
    else:
        stages, res = run_device(per_core, shared, plan, trace=_trace)
    out = host_combine(stages, plan, b2)
    if _trace:
        return out, res
    return out


# revision 44
# speedup vs baseline: 1.9476x; 1.2077x over previous
"""CFConv (SchNet-style continuous-filter convolution) Bass kernel for 8 trn2 cores.

Computation:  f = x@W1;  wf = w_ij * f[idx_j];  conv = segment_sum(wf, seg_i);
              out = conv@W2 + b2

Sharding: edges split equally across 8 cores at segment boundaries. Each core
computes the full node-feature table f = x@W1 (replicated), gathers neighbor
rows with dma_gather, multiplies by w_ij, segment-sums via one-hot matmuls on
the PE (PSUM accumulation over 128-atom windows), applies W2, and writes
per-window partial outputs. Host overlap-adds window outputs (exact: @W2 is
linear) and adds b2.

dma_gather indices are int16, so the f table is split by atom id at 32768
into two HBM tables; each core's edges are processed in two phases (A, B) —
segment-sum linearity makes the split exact. Each table is laid out
partition-major (see _remap_idx) so f-phase chunks write one contiguous
descriptor per partition, and table A completes early so phase-A gathers
overlap the rest of the f-phase. Within each segment window, edges are
sorted by gather row (the one-hot follows the edge, so order is free),
giving the gather ascending DMA addresses. Each block's gather is split
into 4 sub-gathers round-robined over 4 SWDGE queues to parallelize Q7
descriptor generation — the kernel's critical path.

Steady-state timing is measured by wrapping the program in a tc.For_i
hardware loop (bench_device repeat=128), amortizing the ~10ms host/axon
dispatch cost per jitted call.
"""

import math
import os
import sys

import numpy as np

for _p in ("/opt/trn_rl_repo", "/root/.axon_site/_ro/trn_rl_repo"):
    if os.path.isdir(_p) and _p not in sys.path:
        sys.path.insert(0, _p)

import ml_dtypes

BF16 = ml_dtypes.bfloat16
FP8 = ml_dtypes.float8_e4m3
W8 = os.environ.get("KERNEL_W8") == "1"  # stream w_ij as fp8e4m3

# Problem shape (hardcoded per harness contract)
N_ATOMS = 50000
N_EDGES = 1600000
D = 128
N_CORES = 8

TBL_SPLIT = 32768  # int16 gather-index limit

# Block geometry: GPW groups of 128 edges per PSUM window, WPB windows per block
GP = 128  # edges per group (matmul contraction dim)
WPB = 4  # windows per block (PSUM bank = 4*128 fp32 columns)

# dma_gather tuning (see exp_gather.py probes): descriptor generation on the
# Q7 SWDGE path is the kernel's critical path; split each block's gather
# across queues to parallelize generation.
GATHER_QUEUES = int(os.environ.get("GATHER_QUEUES", "4"))
GATHER_SPLIT = int(os.environ.get("GATHER_SPLIT", "4"))  # sub-gathers per block
DMA_SCRATCH = int(os.environ.get("DMA_SCRATCH", "16384"))

N_AP = math.ceil(N_ATOMS / 1024) * 1024  # padded atoms (1024-chunk f-phase)
NB_ROWS = N_AP - TBL_SPLIT  # table-B rows (atoms >= TBL_SPLIT)


def _remap_idx(a):
    """Atom id -> phase-local f-table row (partition-major within each table).

    Table A holds atoms < TBL_SPLIT, table B the rest; each is written
    partition-major (atom x at partition x%128, column x//128 of its table)
    so f-phase chunks land as one contiguous descriptor per partition and
    table A completes before table B starts.
    """
    a = np.asarray(a)
    in_a = a < TBL_SPLIT
    al = np.where(in_a, a, a - TBL_SPLIT)
    nr = np.where(in_a, TBL_SPLIT // 128, NB_ROWS // 128)
    return np.where(in_a, 0, TBL_SPLIT) + (al % 128) * nr + al // 128


def _pick_gpw(spans_ok, phase):
    # spans_ok(gpw, phase) -> bool; prefer big blocks (bounded by SBUF)
    for gpw in (12, 10, 8, 7, 6, 5, 4, 2, 1):
        if spans_ok(gpw, phase):
            return gpw
    raise ValueError("cannot window edges: segment spans too wide even at gpw=1")


def _core_edge_cuts(seg):
    """Split edges into N_CORES ranges at segment boundaries, near-equal sizes."""
    E = len(seg)
    cuts = [0]
    for k in range(1, N_CORES):
        t = k * E // N_CORES
        a = seg[t]
        cut = int(np.searchsorted(seg, a, side="left"))
        cuts.append(max(cut, cuts[-1]))
    cuts.append(E)
    return cuts


def _prep_phase(w, idx_local, seg, gpw):
    """Build device arrays for one (core, phase) edge list.

    w: [n,128] float32 edge filters, idx_local: [n] int64 table-local gather
    rows, seg: [n] int64 global atom ids (sorted). Returns dict with per-block
    tiled arrays, or None if a window span exceeds 128.
    """
    groups = gpw * WPB
    blk = groups * GP
    n = len(seg)
    nblk = max(1, math.ceil(n / blk))
    npad = nblk * blk

    w_pad = np.zeros((npad, D), dtype=np.float32)
    w_pad[:n] = w
    idx_pad = np.zeros(npad, dtype=np.int64)
    idx_pad[:n] = idx_local
    seg_pad = np.zeros(npad, dtype=np.int64)
    seg_pad[:n] = seg

    # window bases + local atom ids
    win_edges = gpw * GP
    nwin = nblk * WPB
    seg_w = seg_pad.reshape(nwin, win_edges)
    bases = seg_w[:, 0].copy()
    # pad tail of the partial window: give pads the window's base so c=0
    if n < npad:
        w_first = n // win_edges
        if n % win_edges:
            base_partial = seg_pad[w_first * win_edges]
            bases[w_first] = base_partial
            seg_pad[n : (w_first + 1) * win_edges] = base_partial
        # fully-padded windows already have seg=0, base=0
    c = seg_pad - np.repeat(bases, win_edges)
    if npad and (c.max() >= 128 or c.min() < 0):
        return None

    # within each window the segment one-hot follows the edge, so edge order
    # is free: sort by gather row for ascending DMA addresses
    for wi in range(nwin):
        sl = slice(wi * win_edges, (wi + 1) * win_edges)
        order = np.argsort(idx_pad[sl], kind="stable")
        w_pad[sl] = w_pad[sl][order]
        idx_pad[sl] = idx_pad[sl][order]
        c[sl] = c[sl][order]

    # tile layouts
    # edge i of block at [i%128 partition, i//128 group]
    wt = (
        w_pad.astype(BF16)
        .reshape(nblk, groups, GP, D)
        .transpose(0, 2, 1, 3)
        .copy()
    )  # [nblk, 128, groups, 128]
    ct = c.astype(BF16).reshape(nblk, groups, GP).transpose(0, 2, 1).copy()
    # idx wrapped: position i = s*16 + p -> [p, s]
    it = (
        idx_pad.astype(np.int16)
        .reshape(nblk, blk // 16, 16)
        .transpose(0, 2, 1)
    )  # [nblk, 16, blk//16]
    it = np.tile(it, (1, 8, 1)).copy()  # replicate to 128 partitions
    bases = bases.reshape(nblk, WPB)
    return dict(wt=wt, ct=ct, it=it, bases=bases, nblk=nblk)


def _zero_blocks(nblk, gpw):
    groups = gpw * WPB
    blk = groups * GP
    return dict(
        wt=np.zeros((nblk, GP, groups, D), dtype=BF16),
        ct=np.zeros((nblk, GP, groups), dtype=BF16),
        it=np.zeros((nblk, 128, blk // 16), dtype=np.int16),
        bases=np.zeros((nblk, WPB), dtype=np.int64),
        nblk=nblk,
    )


def _pad_blocks(ph, nblk, gpw):
    if ph["nblk"] == nblk:
        return ph
    z = _zero_blocks(nblk - ph["nblk"], gpw)
    return dict(
        wt=np.concatenate([ph["wt"], z["wt"]]),
        ct=np.concatenate([ph["ct"], z["ct"]]),
        it=np.concatenate([ph["it"], z["it"]]),
        bases=np.concatenate([ph["bases"], z["bases"]]),
        nblk=nblk,
    )


def prep_inputs(x, w_ij, seg_i, idx_j, W1, W2):
    """Host-side preparation. Returns (per_core_maps, shared, plan)."""
    seg = np.asarray(seg_i, dtype=np.int64)
    idx = np.asarray(idx_j, dtype=np.int64)
    w = np.asarray(w_ij, dtype=np.float32)
    x = np.asarray(x, dtype=np.float32)

    idx2 = _remap_idx(idx)  # f-table rows (partition-major layout)

    cuts = _core_edge_cuts(seg)

    def spans_ok(gpw, phase):
        for k in range(N_CORES):
            lo, hi = cuts[k], cuts[k + 1]
            m = idx2[lo:hi] < TBL_SPLIT
            sel = m if phase == 0 else ~m
            s = seg[lo:hi][sel]
            nw = math.ceil(len(s) / (gpw * GP))
            for wi in range(nw):
                ss = s[wi * gpw * GP : (wi + 1) * gpw * GP]
                if len(ss) and ss[-1] - ss[0] >= 128:
                    return False
        return True

    gpw_a = _pick_gpw(spans_ok, 0)
    gpw_b = _pick_gpw(spans_ok, 1)

    phases = []  # [core][phase] dicts
    for k in range(N_CORES):
        lo, hi = cuts[k], cuts[k + 1]
        m = idx2[lo:hi] < TBL_SPLIT
        pair = []
        for pi, sel in enumerate((m, ~m)):
            e = np.nonzero(sel)[0] + lo
            ph = _prep_phase(
                w[e],
                idx2[e] - (0 if pi == 0 else TBL_SPLIT),
                seg[e],
                gpw_a if pi == 0 else gpw_b,
            )
            assert ph is not None, "span check passed but prep failed"
            pair.append(ph)
        phases.append(pair)

    nblk_a = max(p[0]["nblk"] for p in phases)
    nblk_b = max(p[1]["nblk"] for p in phases)
    nblk = nblk_a + nblk_b

    def _aux_pack(ct, it, groups, blk):
        n = ct.shape[0]
        ab = 2 * groups + blk // 8
        aux = np.zeros((n, 128, ab), dtype=np.uint8)
        aux[:, :, : 2 * groups] = ct.view(np.uint8).reshape(n, 128, -1)
        aux[:, :, 2 * groups :] = it.view(np.uint8).reshape(n, 128, -1)
        return aux

    per_core = []
    all_bases = []
    for k in range(N_CORES):
        pa = _pad_blocks(phases[k][0], nblk_a, gpw_a)
        pb = _pad_blocks(phases[k][1], nblk_b, gpw_b)
        wdt = FP8 if W8 else BF16
        per_core.append(
            dict(
                wt_a=np.ascontiguousarray(pa["wt"].astype(wdt)),
                wt_b=np.ascontiguousarray(pb["wt"].astype(wdt)),
                aux_a=_aux_pack(
                    pa["ct"], pa["it"], gpw_a * WPB, gpw_a * WPB * GP
                ),
                aux_b=_aux_pack(
                    pb["ct"], pb["it"], gpw_b * WPB, gpw_b * WPB * GP
                ),
            )
        )
        all_bases.append(np.concatenate([pa["bases"], pb["bases"]]))

    # shared tensors
    xT = np.zeros((D, N_AP), dtype=BF16)
    xT[:, :N_ATOMS] = x.T.astype(BF16)
    iota = np.broadcast_to(np.arange(GP, dtype=np.float32), (GP, GP)).astype(BF16)
    shared = dict(
        xT=np.ascontiguousarray(xT),
        W1=W1.astype(BF16),
        W2=W2.astype(np.float32),
        iota=np.ascontiguousarray(iota),
    )
    plan = dict(
        gpw_a=gpw_a,
        gpw_b=gpw_b,
        nblk_a=nblk_a,
        nblk_b=nblk_b,
        nblk=nblk,
        bases=all_bases,
    )
    return per_core, shared, plan


def host_combine(stages, plan, b2):
    """stages: list of [NBLK, 128, WPB*128] bf16 outT arrays (per core)."""
    out = np.zeros((N_ATOMS + GP, D), dtype=np.float64)
    for k in range(N_CORES):
        st = np.asarray(stages[k]).astype(np.float64)
        nblk = plan["nblk"]
        # [NBLK, 128do, WPB, 128a] -> [NBLK, WPB, 128a, 128do]
        st = st.reshape(nblk, D, WPB, GP).transpose(0, 2, 3, 1)
        bases = plan["bases"][k]
        for b in range(nblk):
            for wi in range(WPB):
                base = int(bases[b, wi])
                out[base : base + GP] += st[b, wi]
    return (out[:N_ATOMS] + np.asarray(b2, dtype=np.float64)).astype(np.float32)


# ---------------------------------------------------------------------------
# numpy emulation of the device program (for validating the decomposition)
# ---------------------------------------------------------------------------


def emulate_device(per_core, shared, plan, exact=False):
    cast = (lambda a: a.astype(np.float32)) if exact else (
        lambda a: a.astype(BF16).astype(np.float32)
    )
    xT = shared["xT"].astype(np.float32)
    W1 = shared["W1"].astype(np.float32)
    W2 = shared["W2"].astype(np.float32)
    f = cast(xT.T @ W1)  # [N_AP, 128] in atom order (bf16-rounded)
    # partition-major table: row r = (a%128)*NROWS + a//128  ->  f2[r] = f[a]
    a_of_r = np.empty(N_AP, dtype=np.int64)
    r = _remap_idx(np.arange(N_AP))
    a_of_r[r] = np.arange(N_AP)
    f2 = f[a_of_r]
    stages = []
    for k in range(N_CORES):
        m = per_core[k]
        nblk = plan["nblk"]
        stage = np.zeros((nblk, D, WPB * GP), dtype=np.float32)
        for b in range(nblk):
            in_a = b < plan["nblk_a"]
            gpw = plan["gpw_a"] if in_a else plan["gpw_b"]
            groups = gpw * WPB
            blk = groups * GP
            aux = m["aux_a"] if in_a else m["aux_b"]
            wt = m["wt_a"] if in_a else m["wt_b"]
            bl = b if in_a else b - plan["nblk_a"]
            tbl_off = 0 if in_a else TBL_SPLIT
            ct = (
                aux[bl, :, : 2 * groups].copy().view(BF16).astype(np.float32)
            )  # [128, groups]
            it = aux[bl, :, 2 * groups :].copy().view(np.int16)
            idx = it[:16].T.reshape(-1).astype(np.int64)  # [blk] in (s p) order
            w_t = wt[bl].astype(np.float32)  # [128, groups, 128]
            fj = f2[idx + tbl_off].reshape(groups, GP, D).transpose(1, 0, 2)
            wf = cast(w_t * fj)  # [128, groups, 128]
            convT = np.zeros((D, WPB * GP), dtype=np.float32)
            for g in range(groups):
                S = (ct[:, g : g + 1] == np.arange(GP)[None, :]).astype(np.float32)
                wi = g // gpw
                convT[:, wi * GP : (wi + 1) * GP] += wf[:, g, :].T @ S
            stage[b] = cast(W2.T @ convT)
        stages.append(stage.astype(BF16))
    return stages


# ---------------------------------------------------------------------------
# bass device program
# ---------------------------------------------------------------------------


def build_program(plan, repeat=1):
    """Build the device program. With repeat>1 the whole computation runs
    `repeat` times inside a hardware loop (identical work each iteration;
    outputs are rewritten idempotently) so steady-state per-execution time
    can be measured as exec_time/repeat, amortizing host dispatch cost."""
    from contextlib import nullcontext

    import concourse.bacc as bacc
    import concourse.mybir as mybir
    import concourse.tile as tile

    fp32 = mybir.dt.float32
    bf16 = mybir.dt.bfloat16
    i16 = mybir.dt.int16
    u8 = mybir.dt.uint8

    gpw_a = plan["gpw_a"]
    gpw_b = plan["gpw_b"]
    nblk = plan["nblk"]
    nblk_a = plan["nblk_a"]
    nblk_b = plan["nblk_b"]

    def _geom(gpw):
        groups = gpw * WPB
        blk = groups * GP
        return groups, blk, 2 * groups + blk // 8

    groups_a, blk_a, ab_a = _geom(gpw_a)
    groups_b, blk_b, ab_b = _geom(gpw_b)

    nc = bacc.Bacc(
        "TRN2",
        target_bir_lowering=False,
        debug=False,
        num_devices=N_CORES,
        num_swdge_queues=GATHER_QUEUES,
        dynamic_dma_scratch_size=DMA_SCRATCH,
    )

    xT_d = nc.dram_tensor("xT", [D, N_AP], bf16, kind="ExternalInput")
    W1_d = nc.dram_tensor("W1", [D, D], bf16, kind="ExternalInput")
    W2_d = nc.dram_tensor("W2", [D, D], fp32, kind="ExternalInput")
    iota_d = nc.dram_tensor("iota", [GP, GP], bf16, kind="ExternalInput")
    wdt = mybir.dt.float8e4 if W8 else bf16
    wta_d = nc.dram_tensor(
        "wt_a", [nblk_a, GP, groups_a, D], wdt, kind="ExternalInput"
    )
    wtb_d = nc.dram_tensor(
        "wt_b", [nblk_b, GP, groups_b, D], wdt, kind="ExternalInput"
    )
    auxa_d = nc.dram_tensor(
        "aux_a", [nblk_a, 128, ab_a], u8, kind="ExternalInput"
    )
    auxb_d = nc.dram_tensor(
        "aux_b", [nblk_b, 128, ab_b], u8, kind="ExternalInput"
    )
    stage_d = nc.dram_tensor(
        "stage", [nblk, D, WPB * GP], bf16, kind="ExternalOutput"
    )

    with tile.TileContext(nc) as tc:
        with (
            tc.tile_pool(name="consts", bufs=1) as consts,
            tc.tile_pool(name="dram", bufs=1, space="DRAM") as dram_pool,
        ):
            # per-phase f tables, each partition-major (see _remap_idx)
            f_da = dram_pool.tile([TBL_SPLIT, D], bf16)
            f_db = dram_pool.tile([NB_ROWS, D], bf16)
            f_pma = f_da[:].rearrange("(p c) d -> p c d", p=128)
            f_pmb = f_db[:].rearrange("(p c) d -> p c d", p=128)

            W1_sb = consts.tile([D, D], bf16)
            nc.sync.dma_start(W1_sb[:], W1_d[:])
            W2_sb = consts.tile([D, D], fp32)
            nc.sync.dma_start(W2_sb[:], W2_d[:])
            iota_sb = consts.tile([GP, GP], bf16)
            nc.sync.dma_start(iota_sb[:], iota_d[:])

            _ab_nofphase = os.environ.get("KERNEL_NOFPHASE") == "1"
            _ab_nogather = os.environ.get("KERNEL_NOGATHER") == "1"
            _ab_gatheronly = os.environ.get("KERNEL_GATHERONLY") == "1"

            rep_ctx = tc.For_i(0, repeat) if repeat > 1 else nullcontext(0)
            with rep_ctx:
                # ---------------- f-phase: f = x @ W1 ----------------
                CH = 8  # 128-atom tiles per chunk
                nchunks = N_AP // (CH * GP)
                chunk_list = (
                    [0, TBL_SPLIT // (CH * GP)]
                    if _ab_nofphase
                    else range(nchunks)
                )
                with (
                    tc.tile_pool(name="xt", bufs=3) as xt_pool,
                    tc.tile_pool(name="fsb", bufs=3) as fsb_pool,
                    tc.tile_pool(name="fps", bufs=2, space="PSUM") as fps_pool,
                ):
                    for ci in chunk_list:
                        a0 = ci * CH * GP
                        xt = xt_pool.tile([D, CH * GP], bf16)
                        nc.sync.dma_start(xt[:], xT_d[:, a0 : a0 + CH * GP])
                        fps = fps_pool.tile([GP, CH, D], fp32)
                        for i in range(CH):
                            nc.tensor.matmul(
                                fps[:, i, :],
                                xt[:, i * GP : (i + 1) * GP],
                                W1_sb[:],
                                start=True,
                                stop=True,
                            )
                        fsb = fsb_pool.tile([GP, CH, D], bf16)
                        nc.scalar.copy(fsb[:], fps[:])
                        # atom a0+i*128+p -> table row (p, local_col): one
                        # contiguous descriptor per partition
                        ca = TBL_SPLIT // (CH * GP)
                        dst = (
                            f_pma[:, ci * CH : (ci + 1) * CH, :]
                            if ci < ca
                            else f_pmb[:, (ci - ca) * CH : (ci - ca + 1) * CH, :]
                        )
                        nc.sync.dma_start(dst, fsb[:])

                # ---------------- main loop ----------------
                with (
                    tc.tile_pool(name="wsb", bufs=4) as w_pool,
                    tc.tile_pool(name="fj", bufs=4) as fj_pool,
                    tc.tile_pool(name="wf", bufs=2) as wf_pool,
                    tc.tile_pool(name="S", bufs=2) as s_pool,
                    tc.tile_pool(name="aux", bufs=6) as aux_pool,
                    tc.tile_pool(name="cvs", bufs=2) as cvs_pool,
                    tc.tile_pool(name="os", bufs=2) as os_pool,
                    tc.tile_pool(name="cvp", bufs=3, space="PSUM") as cvp_pool,
                    tc.tile_pool(name="otp", bufs=3, space="PSUM") as otp_pool,
                ):
                    gq = 0
                    for b in range(nblk):
                        in_a = b < nblk_a
                        gpw = gpw_a if in_a else gpw_b
                        groups = gpw * WPB
                        blk = groups * GP
                        ab = ab_a if in_a else ab_b
                        bl = b if in_a else b - nblk_a
                        wt_d = wta_d if in_a else wtb_d
                        aux_d = auxa_d if in_a else auxb_d

                        aux_sb = aux_pool.tile([128, ab], u8)
                        nc.sync.dma_start(aux_sb[:], aux_d[bl])
                        w_sb = w_pool.tile([GP, groups, D], wdt)
                        nc.sync.dma_start(w_sb[:], wt_d[bl])
                        ct_sb = aux_sb[:, : 2 * groups].bitcast(bf16)
                        it_sb = aux_sb[:, 2 * groups :].bitcast(i16)

                        fj_sb = fj_pool.tile([GP, groups, D], bf16)
                        tbl = f_da[:] if in_a else f_db[:]
                        if _ab_nogather:
                            nc.vector.memset(fj_sb[:, 0, :], 0.0)
                        else:
                            ns = GATHER_SPLIT
                            n_i = blk // ns
                            for h in range(ns):
                                nc.gpsimd.dma_gather(
                                    fj_sb[:, h * (groups // ns) :
                                          (h + 1) * (groups // ns), :],
                                    tbl,
                                    it_sb[:, h * (n_i // 16) :
                                          (h + 1) * (n_i // 16)],
                                    n_i,
                                    n_i,
                                    D,
                                    single_packet=False,
                                    queue_num=gq % GATHER_QUEUES,
                                )
                                gq += 1

                        if _ab_gatheronly:
                            osb = os_pool.tile([D, WPB * GP], bf16)
                            nc.vector.memset(osb[:, 0:4], 0.0)
                            nc.sync.dma_start(stage_d[b], osb[:])
                            continue

                        wf_sb = wf_pool.tile([GP, groups, D], bf16)
                        nc.vector.tensor_mul(wf_sb[:], w_sb[:], fj_sb[:])

                        s_sb = s_pool.tile([GP, groups, D], bf16)
                        nc.vector.tensor_tensor(
                            s_sb[:],
                            ct_sb.unsqueeze(2).broadcast_to([GP, groups, GP]),
                            iota_sb[:].unsqueeze(1).broadcast_to(
                                [GP, groups, GP]
                            ),
                            mybir.AluOpType.is_equal,
                        )

                        cvp = cvp_pool.tile([D, WPB, GP], fp32)
                        for g in range(groups):
                            wi = g // gpw
                            nc.tensor.matmul(
                                cvp[:, wi, :],
                                wf_sb[:, g, :],
                                s_sb[:, g, :],
                                start=(g % gpw == 0),
                                stop=(g % gpw == gpw - 1),
                            )
                        cvs = cvs_pool.tile([D, WPB * GP], fp32)
                        nc.scalar.copy(
                            cvs[:], cvp[:].rearrange("d w a -> d (w a)")
                        )

                        otp = otp_pool.tile([D, WPB * GP], fp32)
                        nc.tensor.matmul(
                            otp[:], W2_sb[:], cvs[:], start=True, stop=True
                        )
                        osb = os_pool.tile([D, WPB * GP], bf16)
                        nc.scalar.copy(osb[:], otp[:])
                        nc.sync.dma_start(stage_d[b], osb[:])

    nc.compile()
    return nc


def run_device(per_core, shared, plan, trace=False):
    from concourse import bass_utils

    nc = build_program(plan)
    in_maps = []
    for k in range(N_CORES):
        m = dict(shared)
        m.update(per_core[k])
        in_maps.append(
            {
                "xT": np.ascontiguousarray(m["xT"]),
                "W1": np.ascontiguousarray(m["W1"]),
                "W2": np.ascontiguousarray(m["W2"]),
                "iota": np.ascontiguousarray(m["iota"]),
                "wt_a": np.ascontiguousarray(m["wt_a"]),
                "wt_b": np.ascontiguousarray(m["wt_b"]),
                "aux_a": np.ascontiguousarray(m["aux_a"]),
                "aux_b": np.ascontiguousarray(m["aux_b"]),
            }
        )
    res = bass_utils.run_bass_kernel_spmd(
        nc, in_maps, core_ids=list(range(N_CORES)), trace=trace
    )
    stages = [r["stage"] for r in res.results]
    return stages, res


def bench_device(per_core, shared, plan, repeat=128, nbatches=4):
    """Steady-state per-execution device time.

    The device program repeats the full computation `repeat` times inside a
    hardware loop (see build_program), so one NEFF execution performs
    `repeat` back-to-back runs. Per-run time = call_time / repeat; the ~10ms
    host/axon dispatch overhead amortizes to noise.
    """
    import time

    import jax
    from jax.sharding import Mesh, PartitionSpec
    from jax.experimental.shard_map import shard_map
    from concourse.bass2jax import (
        _bass_exec_p,
        install_neuronx_cc_hook,
        partition_id_tensor,
    )
    import concourse.mybir as mybir

    repeat = int(os.environ.get("BENCH_REPEAT", repeat))
    install_neuronx_cc_hook()
    nc = build_program(plan, repeat=repeat)
    partition_name = (
        nc.partition_id_tensor.name if nc.partition_id_tensor else None
    )

    in_names = []
    out_names = []
    out_avals = []
    zero_outs = []
    for alloc in nc.m.functions[0].allocations:
        if not isinstance(alloc, mybir.MemoryLocationSet):
            continue
        name = alloc.memorylocations[0].name
        if alloc.kind == "ExternalInput":
            if name != partition_name:
                in_names.append(name)
        elif alloc.kind == "ExternalOutput":
            out_names.append(name)
            dt = mybir.dt.np(alloc.dtype)
            out_avals.append(
                jax.core.ShapedArray(tuple(alloc.tensor_shape), dt)
            )
            zero_outs.append(np.zeros(tuple(alloc.tensor_shape), dt))
    n_params = len(in_names)
    all_names = in_names + out_names
    if partition_name is not None:
        all_names = all_names + [partition_name]

    def _body(*args):
        operands = list(args)
        if partition_name is not None:
            operands.append(partition_id_tensor())
        outs = _bass_exec_p.bind(
            *operands,
            out_avals=tuple(out_avals),
            in_names=tuple(all_names),
            out_names=tuple(out_names),
            lowering_input_output_aliases=(),
            sim_require_finite=True,
            sim_require_nnan=True,
            nc=nc,
        )
        return tuple(outs)

    devices = jax.devices()[:N_CORES]
    mesh = Mesh(np.asarray(devices), ("core",))
    nin = n_params + len(zero_outs)
    sharded = jax.jit(
        shard_map(
            _body,
            mesh=mesh,
            in_specs=(PartitionSpec("core"),) * nin,
            out_specs=(PartitionSpec("core"),) * len(out_names),
            check_rep=False,
        ),
        keep_unused=True,
    )

    in_maps = []
    for k in range(N_CORES):
        m = dict(shared)
        m.update(per_core[k])
        in_maps.append(m)
    concat = [
        np.concatenate([np.asarray(in_maps[c][n]) for c in range(N_CORES)], axis=0)
        for n in in_names
    ] + [np.zeros((N_CORES * z.shape[0], *z.shape[1:]), z.dtype) for z in zero_outs]
    from jax.sharding import NamedSharding

    sh = NamedSharding(mesh, PartitionSpec("core"))
    dev_in = [jax.device_put(a, sh) for a in concat]

    # warmup (compile + first run)
    out = sharded(*dev_in)
    jax.block_until_ready(out)
    t0 = time.perf_counter()
    out = sharded(*dev_in)
    jax.block_until_ready(out)
    single = (time.perf_counter() - t0) / repeat
    times = []
    for _ in range(nbatches):
        tb = time.perf_counter()
        out2 = sharded(*dev_in)
        jax.block_until_ready(out2)
        times.append((time.perf_counter() - tb) / repeat)
    per_iter = min(times)
    stage_g = np.asarray(out[0]).reshape(N_CORES, *out_avals[0].shape)
    stages = [stage_g[c] for c in range(N_CORES)]
    return stages, dict(single_s=single, per_iter_s=per_iter)


def kernel(x, w_ij, seg_i, idx_j, seg_i_sum, W1, W2, b2, _trace=False, _emulate=False):
    per_core, shared, plan = prep_inputs(x, w_ij, seg_i, idx_j, W1, W2)
    if _emulate:
        stages = emulate_device(per_core, shared, plan)
        res = None
    else:
        stages, res = run_device(per_core, shared, plan, trace=_trace)
    out = host_combine(stages, plan, b2)
    if _trace:
        return out, res
    return out
